# revision 29
# baseline (speedup 1.0000x reference)
import sys

sys.path.insert(0, '/opt/trn_rl_repo')
import numpy as np
import concourse.bacc as bacc
import concourse.mybir as mybir
import concourse.tile as tile
from concourse.bass_utils import run_bass_kernel_spmd
from concourse.masks import make_identity

dt = mybir.dt
F32 = dt.float32
F32R = dt.float32r
Alu = mybir.AluOpType
Act = mybir.ActivationFunctionType
AX = mybir.AxisListType

BS, N, B, D, M = 4, 1024, 8, 64, 2048
NT, MT = N // 128, M // 128          # 8 n-tiles, 16 m-tiles
NQ = 4                               # n-quarters (2 n-tiles each)
PAIRS = BS * B
NCORES = 8
PPC = PAIRS // NCORES
C = 64
S_MAX = 4.0
BUDGET = 512.0
NEG = -1.0e9
EPS = 1e-8


def build_program(debug=False):
    nc = bacc.Bacc("TRN2", target_bir_lowering=False, debug=False)

    q_d = nc.dram_tensor("q", [PPC, N, D], F32, kind="ExternalInput").ap()
    qn_d = nc.dram_tensor("qn", [PPC, N, D], F32, kind="ExternalInput").ap()
    vn_d = nc.dram_tensor("vn", [PPC, N, D], F32, kind="ExternalInput").ap()
    sur_d = nc.dram_tensor("sur", [PPC, N], F32, kind="ExternalInput").ap()
    wn_d = nc.dram_tensor("wn", [PPC, N], F32, kind="ExternalInput").ap()
    gtdw_d = nc.dram_tensor("gtdw", [PPC, 4], F32, kind="ExternalInput").ap()
    emK_d = nc.dram_tensor("emK", [PPC, M, D], F32, kind="ExternalInput").ap()
    emV_d = nc.dram_tensor("emV", [PPC, M, D], F32, kind="ExternalInput").ap()
    emS_d = nc.dram_tensor("emS", [PPC, M], F32, kind="ExternalInput").ap()
    emA_d = nc.dram_tensor("emA", [PPC, M], F32, kind="ExternalInput").ap()

    out_d = nc.dram_tensor("out", [PPC, N, D], F32, kind="ExternalOutput").ap()
    nK_d = nc.dram_tensor("nK", [PPC, M, D], F32, kind="ExternalOutput").ap()
    nV_d = nc.dram_tensor("nV", [PPC, M, D], F32, kind="ExternalOutput").ap()
    nS_d = nc.dram_tensor("nS", [PPC, M], F32, kind="ExternalOutput").ap()
    nA_d = nc.dram_tensor("nA", [PPC, M], F32, kind="ExternalOutput").ap()
    if debug:
        dbg_v16 = nc.dram_tensor("dbg_v16", [PPC, N, 16], F32, kind="ExternalOutput").ap()
        dbg_nov = nc.dram_tensor("dbg_nov", [PPC, N], F32, kind="ExternalOutput").ap()
        dbg_cand = nc.dram_tensor("dbg_cand", [PPC, C], F32, kind="ExternalOutput").ap()
        dbg_ckv = nc.dram_tensor("dbg_ckv", [PPC, C, 128], F32, kind="ExternalOutput").ap()
        dbg_alpha = nc.dram_tensor("dbg_alpha", [PPC, C, M], F32, kind="ExternalOutput").ap()

    with tile.TileContext(nc) as tc:
        with (
            tc.tile_pool(name="const", bufs=1) as cpool,
            tc.tile_pool(name="persist", bufs=PPC) as pp,
            tc.tile_pool(name="rt1", bufs=1) as rt1,
            tc.tile_pool(name="rtA", bufs=2) as rtA,
            tc.tile_pool(name="rtW", bufs=2) as rtW,
            tc.tile_pool(name="rtK", bufs=2) as rtK,
            tc.tile_pool(name="rtB2", bufs=2) as rtB2,
            tc.tile_pool(name="rtC", bufs=3) as rtC,
            tc.tile_pool(name="sm", bufs=3) as sm,
            tc.tile_pool(name="novTp", bufs=2) as novT_pool,
            tc.tile_pool(name="psS", bufs=2, space="PSUM") as poolS,
            tc.tile_pool(name="psM", bufs=2, space="PSUM") as poolM,
            tc.tile_pool(name="psO", bufs=2, space="PSUM") as poolO,
            tc.tile_pool(name="psX", bufs=2, space="PSUM") as poolX,
        ):
            ident = cpool.tile([128, 128], F32)
            make_identity(nc, ident[:])
            ones128 = cpool.tile([128, 1], F32)
            nc.vector.memset(ones128[:], 1.0)
            ones1x128 = cpool.tile([1, 128], F32)
            nc.vector.memset(ones1x128[:], 1.0)

            # ---------- per-pair persistent tiles ----------
            P_emK, P_qn, P_vn = [], [], []
            P_sur, P_wn, P_sim, P_nov = [], [], [], []
            P_S16, P_A16, P_gtdw = [], [], []
            for p in range(PPC):
                emK_nat = pp.tile([128, MT * D], F32, tag="emK")
                nc.sync.dma_start(emK_nat[:].rearrange("q (t d) -> q t d", t=MT),
                                  emK_d[p].rearrange("(t q) d -> q t d", q=128))
                qn_nat = pp.tile([128, NT * D], F32, tag="qn")
                nc.sync.dma_start(qn_nat[:].rearrange("q (t d) -> q t d", t=NT),
                                  qn_d[p].rearrange("(t q) d -> q t d", q=128))
                vn_nat = pp.tile([128, NT * D], F32, tag="vn")
                nc.sync.dma_start(vn_nat[:].rearrange("q (t d) -> q t d", t=NT),
                                  vn_d[p].rearrange("(t q) d -> q t d", q=128))
                sur_t = pp.tile([128, NT], F32, tag="sur")
                nc.sync.dma_start(sur_t[:], sur_d[p].rearrange("(t q) -> q t", q=128))
                wn_t = pp.tile([128, NT], F32, tag="wn")
                nc.sync.dma_start(wn_t[:], wn_d[p].rearrange("(t q) -> q t", q=128))
                S16 = pp.tile([128, MT], F32, tag="S16")
                nc.sync.dma_start(S16[:], emS_d[p].rearrange("(t q) -> q t", q=128))
                A16 = pp.tile([128, MT], F32, tag="A16")
                nc.sync.dma_start(A16[:], emA_d[p].rearrange("(t q) -> q t", q=128))
                gt = pp.tile([1, 4], F32, tag="gtdw")
                nc.sync.dma_start(gt[:], gtdw_d[p].rearrange("(a c) -> a c", a=1))
                simmax = pp.tile([128, NT], F32, tag="simmax")
                nov_sb = pp.tile([128, NT], F32, tag="nov")
                P_emK.append(emK_nat)
                P_qn.append(qn_nat); P_vn.append(vn_nat)
                P_sur.append(sur_t); P_wn.append(wn_t)
                P_sim.append(simmax); P_nov.append(nov_sb)
                P_S16.append(S16); P_A16.append(A16); P_gtdw.append(gt)

            cand = cpool.tile([PPC, C], F32)
            csum = cpool.tile([PPC, 1], F32)

            # =================== READ PHASE ===================
            for p in range(PPC):
                q_nat = rt1.tile([128, NT * D], F32, tag="q_nat")
                nc.sync.dma_start(q_nat[:].rearrange("q (t d) -> q t d", t=NT),
                                  q_d[p].rearrange("(t q) d -> q t d", q=128))
                qT = rt1.tile([64, N], F32, tag="qT")
                qnT = rt1.tile([64, N], F32, tag="qnT")
                for t in range(NT):
                    psT = poolX.tile([128, 130], F32, tag="tp")
                    nc.tensor.transpose(psT[:64, :128], q_nat[:, t * D:(t + 1) * D], ident[:])
                    nc.scalar.copy(qT[:, t * 128:(t + 1) * 128], psT[:64, :128])
                    psT2 = poolX.tile([128, 130], F32, tag="tp")
                    nc.tensor.transpose(psT2[:64, :128], P_qn[p][:, t * D:(t + 1) * D], ident[:])
                    nc.scalar.copy(qnT[:, t * 128:(t + 1) * 128], psT2[:64, :128])

                KTs = rt1.tile([64, M], F32, tag="KTs")
                for t in range(MT):
                    psT = poolX.tile([128, 130], F32, tag="tp")
                    nc.tensor.transpose(psT[:64, :128], P_emK[p][:, t * D:(t + 1) * D], ident[:])
                    nc.scalar.copy(KTs[:, t * 128:(t + 1) * 128], psT[:64, :128])

                # V65r: [128, MT*65] fp32r: emV blocks + ones col
                V65f = rt1.tile([128, MT * 65], F32, tag="V65f")
                nc.sync.dma_start(
                    V65f[:].rearrange("q (t d) -> q t d", t=MT)[:, :, 0:D],
                    emV_d[p].rearrange("(t q) d -> q t d", q=128))
                nc.vector.memset(V65f[:].rearrange("q (t d) -> q t d", t=MT)[:, :, D:65], 1.0)
                V65r = rt1.tile([128, MT * 65], F32R, tag="V65r")
                nc.scalar.copy(V65r[:], V65f[:])

                def flush_tile(j, e_tile, wT_cur):
                    nq = j // 2
                    ii = j % 2
                    for t in range(MT):
                        psT = poolX.tile([128, 130], F32, tag="tp")
                        nc.tensor.transpose(psT[:128, :128],
                                            e_tile[:, t * 128:(t + 1) * 128], ident[:])
                        nc.scalar.copy(
                            wT_cur[:, t * 256 + ii * 128: t * 256 + (ii + 1) * 128],
                            psT[:128, :128])
                    if ii == 0:
                        return
                    psO = poolO.tile([65, 256], F32, tag="outmm")
                    for t in range(MT):
                        nc.tensor.matmul(
                            psO[:], V65r[:, t * 65:(t + 1) * 65],
                            wT_cur[:, t * 256:(t + 1) * 256],
                            start=(t == 0), stop=(t == MT - 1))
                    outT = sm.tile([65, 256], F32, tag="outT")
                    nc.scalar.copy(outT[:], psO[:])
                    outF = sm.tile([128, 130], F32, tag="outF")
                    for jj in range(2):
                        psT = poolX.tile([128, 130], F32, tag="tp")
                        nc.tensor.transpose(psT[:128, :65],
                                            outT[:, jj * 128:(jj + 1) * 128], ident[:65, :65])
                        nc.scalar.copy(outF[:, jj * 65:(jj + 1) * 65], psT[:128, :65])
                    den2 = sm.tile([128, 2], F32, tag="den2")
                    nc.vector.tensor_copy(
                        den2[:], outF[:].rearrange("q (t x) -> q t x", t=2)[:, :, 64:65])
                    rec2 = sm.tile([128, 2], F32, tag="rec2")
                    nc.vector.reciprocal(rec2[:], den2[:])
                    outfin = sm.tile([128, 2 * D], F32, tag="outfin")
                    for jj in range(2):
                        nc.vector.tensor_scalar_mul(
                            outfin[:, jj * D:(jj + 1) * D],
                            outF[:, jj * 65: jj * 65 + 64], rec2[:, jj:jj + 1])
                    nc.sync.dma_start(
                        out_d[p].rearrange("(t q) d -> q t d", q=128)[:, nq * 2:(nq + 1) * 2, :],
                        outfin[:].rearrange("q (t d) -> q t d", t=2))

                pend = None
                wT_cur = None
                for i in range(NT):
                    if i % 2 == 0:
                        wT_next = rtW.tile([128, MT * 256], F32R, tag="wT")
                    s_sb = rtA.tile([128, M], F32, tag="bigA")
                    for h in range(4):
                        ps = poolS.tile([128, 512], F32, tag="mmS")
                        nc.tensor.matmul(
                            ps[:], qT[:, i * 128:(i + 1) * 128],
                            KTs[:, h * 512:(h + 1) * 512], start=True, stop=True)
                        nc.scalar.copy(s_sb[:, h * 512:(h + 1) * 512], ps[:])
                    smax4 = sm.tile([128, 4], F32, tag="smax4")
                    for h in range(4):
                        ps = poolM.tile([128, 512], F32, tag="mmM")
                        nc.tensor.matmul(
                            ps[:], qnT[:, i * 128:(i + 1) * 128],
                            KTs[:, h * 512:(h + 1) * 512], start=True, stop=True)
                        nc.vector.reduce_max(smax4[:, h:h + 1], ps[:], axis=AX.X)
                    nc.vector.reduce_max(P_sim[p][:, i:i + 1], smax4[:], axis=AX.X)

                    v18 = sm.tile([128, 8], F32, tag="v18")
                    nc.vector.max(out=v18[:], in_=s_sb[:])
                    s2 = rtC.tile([128, M], F32, tag="bigC")
                    nc.vector.match_replace(out=s2[:], in_to_replace=v18[:],
                                            in_values=s_sb[:], imm_value=NEG)
                    v916 = sm.tile([128, 8], F32, tag="v916")
                    nc.vector.max(out=v916[:], in_=s2[:])
                    if debug:
                        v16c = sm.tile([128, 16], F32, tag="v16c")
                        nc.vector.tensor_copy(v16c[:, 0:8], v18[:])
                        nc.vector.tensor_copy(v16c[:, 8:16], v916[:])
                        nc.sync.dma_start(
                            dbg_v16[p].rearrange("(t q) k -> q t k", q=128)[:, i:i + 1, :],
                            v16c[:])
                    tstar = v916[:, 7:8]
                    tneg = sm.tile([128, 1], F32, tag="tneg")
                    nc.vector.tensor_scalar_mul(tneg[:], v18[:, 0:1], -1.0)
                    e_sb = rtC.tile([128, M], F32, tag="bigC")
                    nc.scalar.activation(e_sb[:], s_sb[:], Act.Exp, bias=tneg[:], scale=1.0)
                    nc.vector.tensor_scalar(
                        s_sb[:], s_sb[:], tstar, None, op0=Alu.is_ge)
                    nc.gpsimd.tensor_tensor(
                        out=e_sb[:], in0=e_sb[:], in1=s_sb[:], op=Alu.mult)
                    if pend is not None:
                        flush_tile(pend[0], pend[1], pend[2])
                    pend = (i, e_sb, wT_next)
                    wT_cur = wT_next
                flush_tile(pend[0], pend[1], pend[2])

                # novelty for this pair
                ms = sm.tile([128, NT], F32, tag="ms")
                nc.vector.tensor_scalar_max(ms[:], P_sim[p][:], 0.0)
                om = sm.tile([128, NT], F32, tag="om")
                nc.vector.tensor_scalar(om[:], ms[:], -1.0, 1.0, op0=Alu.mult, op1=Alu.add)
                ow = sm.tile([128, NT], F32, tag="ow")
                nc.vector.tensor_scalar(ow[:], P_wn[p][:], -1.0, 1.0, op0=Alu.mult, op1=Alu.add)
                nc.vector.tensor_tensor(out=om[:], in0=om[:], in1=ow[:], op=Alu.mult)
                nc.vector.tensor_tensor(out=ow[:], in0=P_wn[p][:], in1=P_sur[p][:], op=Alu.mult)
                nc.vector.tensor_tensor(out=P_nov[p][:], in0=om[:], in1=ow[:], op=Alu.add)
                if debug:
                    nc.sync.dma_start(dbg_nov[p].rearrange("(t q) -> q t", q=128), P_nov[p][:])

            # =================== TOP-64 BATCH ===================
            novT = novT_pool.tile([PPC, N], F32, tag="novT")
            for p in range(PPC):
                for t in range(NT):
                    nc.sync.dma_start(
                        novT[p:p + 1, t * 128:(t + 1) * 128],
                        P_nov[p][:, t:t + 1])
            cur = novT
            for r in range(C // 8):
                nc.vector.max(out=cand[:, r * 8:(r + 1) * 8], in_=cur[:])
                if r < C // 8 - 1:
                    nxt = novT_pool.tile([PPC, N], F32, tag="novT")
                    nc.vector.match_replace(out=nxt[:], in_to_replace=cand[:, r * 8:(r + 1) * 8],
                                            in_values=cur[:], imm_value=NEG)
                    cur = nxt
            nc.vector.reduce_sum(csum[:], cand[:], axis=AX.X)
            if debug:
                nc.sync.dma_start(dbg_cand, cand[:])

            # =================== WRITE PHASE ===================
            for p in range(PPC):
                g_ap = P_gtdw[p][0:1, 0:1]
                tau_ap = P_gtdw[p][0:1, 1:2]
                dec_ap = P_gtdw[p][0:1, 2:3]
                ww_ap = P_gtdw[p][0:1, 3:4]

                candp0 = sm.tile([1, C], F32, tag="candp0")
                nc.sync.dma_start(candp0[:], cand[p:p + 1, :])
                csump0 = sm.tile([1, 1], F32, tag="csump0")
                nc.sync.dma_start(csump0[:], csum[p:p + 1, :])
                psB = poolX.tile([128, 130], F32, tag="tp")
                nc.tensor.matmul(psB[:128, 0:C], ones1x128[:], candp0[:],
                                 start=True, stop=True)
                candB = sm.tile([128, C], F32, tag="candB")
                nc.scalar.copy(candB[:], psB[:128, 0:C])
                psC1 = poolX.tile([128, 130], F32, tag="tp")
                nc.tensor.matmul(psC1[:C, 0:1], candp0[:], ones1x128[:, 0:1],
                                 start=True, stop=True)
                candcol = sm.tile([C, 1], F32, tag="candcol")
                nc.scalar.copy(candcol[:], psC1[:C, 0:1])

                ohT = rt1.tile([128, NT * C], F32, tag="q_nat")
                for i in range(NT):
                    nc.vector.tensor_scalar(
                        ohT[:, i * C:(i + 1) * C], candB[:], P_nov[p][:, i:i + 1], None,
                        op0=Alu.is_equal)
                psC = poolX.tile([128, 130], F32, tag="tp")
                for i in range(NT):
                    nc.tensor.matmul(psC[:C, 0:D], ohT[:, i * C:(i + 1) * C],
                                     P_qn[p][:, i * D:(i + 1) * D],
                                     start=(i == 0), stop=(i == NT - 1),
                                     skip_group_check=True)
                for i in range(NT):
                    nc.tensor.matmul(psC[:C, D:2 * D], ohT[:, i * C:(i + 1) * C],
                                     P_vn[p][:, i * D:(i + 1) * D],
                                     start=(i == 0), stop=(i == NT - 1),
                                     skip_group_check=True)
                ckv = sm.tile([C, 2 * D], F32, tag="ckv")
                nc.scalar.copy(ckv[:], psC[:C, 0:2 * D])
                if debug:
                    nc.sync.dma_start(dbg_ckv[p], ckv[:])

                sq = sm.tile([C, D], F32, tag="sqck")
                nrm2 = sm.tile([C, 1], F32, tag="nrm2")
                nc.vector.scalar_tensor_tensor(
                    out=sq[:], in0=ckv[:, 0:D], scalar=1.0, in1=ckv[:, 0:D],
                    op0=Alu.mult, op1=Alu.mult, accum_out=nrm2[:])
                rinv = sm.tile([C, 1], F32, tag="rinv")
                nc.vector.reciprocal(rinv[:], nrm2[:])
                rn = sm.tile([C, 1], F32, tag="rn")
                nc.scalar.activation(rn[:], rinv[:], Act.Sqrt)
                bl_rhs = sm.tile([C, 129], F32, tag="bl_rhs")
                nc.vector.tensor_scalar_mul(bl_rhs[:, 0:D], ckv[:, 0:D], rn[:])
                nc.scalar.copy(bl_rhs[:, D:2 * D], ckv[:, D:2 * D])
                nc.vector.memset(bl_rhs[:, 128:129], 1.0)

                ckT65 = sm.tile([65, C], F32, tag="ckT65")
                psT = poolX.tile([128, 130], F32, tag="tp")
                nc.tensor.transpose(psT[:D, 0:C], bl_rhs[:, 0:D], ident[:C, :C])
                nc.scalar.copy(ckT65[0:D, :], psT[:D, 0:C])
                negww = sm.tile([1, 1], F32, tag="negww")
                nc.vector.tensor_scalar_mul(negww[:], ww_ap, -1.0)
                psW = poolX.tile([128, 130], F32, tag="tp")
                nc.tensor.matmul(psW[:1, 0:C], negww[:], ones1x128[:, 0:C],
                                 start=True, stop=True)
                nc.scalar.copy(ckT65[64:65, :], psW[:1, 0:C])

                KTf = rtK.tile([65, M], F32, tag="KTf")
                for t in range(MT):
                    psT = poolX.tile([128, 130], F32, tag="tp")
                    nc.tensor.transpose(psT[:64, :128], P_emK[p][:, t * D:(t + 1) * D], ident[:])
                    nc.scalar.copy(KTf[0:64, t * 128:(t + 1) * 128], psT[:64, :128])
                sSrow = rt1.tile([1, M], F32, tag="sSrow")
                nc.sync.dma_start(sSrow[:], emS_d[p].rearrange("(a m) -> a m", a=1))
                nc.scalar.copy(KTf[64:65, :], sSrow[:])

                invtau1 = sm.tile([1, 1], F32, tag="invtau1")
                nc.vector.tensor_scalar_max(invtau1[:], tau_ap, 0.01)
                nc.vector.reciprocal(invtau1[:], invtau1[:])
                psI = poolX.tile([128, 130], F32, tag="tp")
                nc.tensor.matmul(psI[:C, 0:1], ones1x128[:, 0:C], invtau1[:],
                                 start=True, stop=True)
                invtau = sm.tile([C, 1], F32, tag="invtau")
                nc.scalar.copy(invtau[:], psI[:C, 0:1])

                slotraw = rtA.tile([C, M], F32, tag="bigA")
                for h in range(4):
                    psL = poolM.tile([128, 512], F32, tag="mmM")
                    nc.tensor.matmul(
                        psL[:C, :], ckT65[:], KTf[:, h * 512:(h + 1) * 512],
                        start=True, stop=True)
                    nc.scalar.copy(slotraw[:, h * 512:(h + 1) * 512], psL[:C, :])
                sw = rtC.tile([C, M], F32, tag="bigC")
                rmax = sm.tile([C, 1], F32, tag="rmax")
                nc.vector.reduce_max(rmax[:], slotraw[:], axis=AX.X)
                nbias = sm.tile([C, 1], F32, tag="nbias")
                nc.vector.tensor_tensor(out=nbias[:], in0=rmax[:], in1=invtau[:], op=Alu.mult)
                nc.vector.tensor_scalar_mul(nbias[:], nbias[:], -1.0)
                rsum = sm.tile([C, 1], F32, tag="rsum")
                nc.scalar.activation(sw[:], slotraw[:], Act.Exp, bias=nbias[:],
                                     scale=invtau[:], accum_out=rsum[:])

                gs1 = sm.tile([1, 1], F32, tag="gs1")
                nc.vector.tensor_scalar_add(gs1[:], csump0[:], EPS)
                nc.vector.reciprocal(gs1[:], gs1[:])
                nc.vector.tensor_tensor(out=gs1[:], in0=gs1[:], in1=g_ap, op=Alu.mult)
                psG = poolX.tile([128, 130], F32, tag="tp")
                nc.tensor.matmul(psG[:C, 0:1], ones1x128[:, 0:C], gs1[:], start=True, stop=True)
                gsC = sm.tile([C, 1], F32, tag="gsC")
                nc.scalar.copy(gsC[:], psG[:C, 0:1])
                alphacol = sm.tile([C, 1], F32, tag="alphacol")
                nc.vector.tensor_scalar_mul(alphacol[:], candcol[:], gsC[:])
                rr = sm.tile([C, 1], F32, tag="rr")
                nc.vector.reciprocal(rr[:], rsum[:])
                nc.vector.tensor_tensor(out=alphacol[:], in0=alphacol[:], in1=rr[:], op=Alu.mult)
                nc.vector.tensor_scalar_mul(sw[:], sw[:], alphacol[:])
                alpha = sw
                if debug:
                    nc.sync.dma_start(dbg_alpha[p], alpha[:])

                blKV = rtB2.tile([128, MT * 129], F32, tag="blKV")
                for t in range(MT):
                    psB2 = poolX.tile([128, 130], F32, tag="tp")
                    nc.tensor.matmul(psB2[:128, 0:129], alpha[:, t * 128:(t + 1) * 128],
                                     bl_rhs[:], start=True, stop=True)
                    nc.scalar.copy(blKV[:, t * 129:(t + 1) * 129], psB2[:128, 0:129])

                aps16 = sm.tile([128, MT], F32, tag="aps16")
                nc.vector.tensor_copy(
                    aps16[:], blKV[:].rearrange("q (t x) -> q t x", t=MT)[:, :, 128:129])
                masku = sm.tile([128, MT], F32, tag="masku")
                nc.vector.tensor_scalar(masku[:], aps16[:], EPS, None, op0=Alu.is_gt)
                blKview = blKV[:].rearrange("q (t x) -> q t x", t=MT)[:, :, 0:D]
                blVview = blKV[:].rearrange("q (t x) -> q t x", t=MT)[:, :, D:2 * D]
                sqb = rt1.tile([128, MT * D], F32, tag="sqb")
                nc.gpsimd.tensor_tensor(out=sqb[:].rearrange("q (t d) -> q t d", t=MT),
                                        in0=blKview, in1=blKview, op=Alu.mult)
                nrm2b = sm.tile([128, MT], F32, tag="nrm2b")
                nc.vector.reduce_sum(nrm2b[:], sqb[:].rearrange("q (t d) -> q t d", t=MT),
                                     axis=AX.X)
                nc.vector.tensor_scalar_max(nrm2b[:], nrm2b[:], 1e-30)
                rnb = sm.tile([128, MT], F32, tag="rnb")
                nc.vector.reciprocal(rnb[:], nrm2b[:])
                nc.scalar.activation(rnb[:], rnb[:], Act.Sqrt)
                nc.vector.tensor_tensor(out=rnb[:], in0=rnb[:], in1=masku[:], op=Alu.mult)
                aeff = sm.tile([128, MT], F32, tag="aeff")
                nc.vector.tensor_scalar_min(aeff[:], aps16[:], 1.0)
                nc.vector.tensor_tensor(out=aeff[:], in0=aeff[:], in1=masku[:], op=Alu.mult)
                onema = sm.tile([128, MT], F32, tag="onema")
                nc.vector.tensor_scalar(onema[:], aeff[:], -1.0, 1.0, op0=Alu.mult, op1=Alu.add)
                scalK = sm.tile([128, MT], F32, tag="scalK")
                nc.vector.tensor_tensor(out=scalK[:], in0=aeff[:], in1=rnb[:], op=Alu.mult)

                nKt = rtA.tile([128, MT * D], F32, tag="bigA")
                nVt = rtC.tile([128, MT * D], F32, tag="bigC")
                onema_b = onema[:].to_broadcast([128, MT, D])
                scalK_b = scalK[:].to_broadcast([128, MT, D])
                nKv = nKt[:].rearrange("q (t d) -> q t d", t=MT)
                nVv = nVt[:].rearrange("q (t d) -> q t d", t=MT)
                emKv = P_emK[p][:].rearrange("q (t d) -> q t d", t=MT)
                emVw = rt1.tile([128, MT * D], F32, tag="emVw")
                nc.sync.dma_start(emVw[:].rearrange("q (t d) -> q t d", t=MT),
                                  emV_d[p].rearrange("(t q) d -> q t d", q=128))
                emVv = emVw[:].rearrange("q (t d) -> q t d", t=MT)
                nc.gpsimd.tensor_tensor(out=nKv, in0=emKv, in1=onema_b, op=Alu.mult)
                sqb2 = rtC.tile([128, MT * D], F32, tag="bigC")
                nc.gpsimd.tensor_tensor(out=sqb2[:].rearrange("q (t d) -> q t d", t=MT),
                                        in0=blKview, in1=scalK_b, op=Alu.mult)
                nc.gpsimd.tensor_tensor(out=nKt[:], in0=nKt[:], in1=sqb2[:], op=Alu.add)
                nc.sync.dma_start(nK_d[p].rearrange("(t q) d -> q t d", q=128),
                                  nKt[:].rearrange("q (t d) -> q t d", t=MT))
                # new_V: scale = aeff / max(aps, eps)
                rdb = sm.tile([128, MT], F32, tag="rdb")
                nc.vector.tensor_scalar_max(rdb[:], aps16[:], EPS)
                nc.vector.reciprocal(rdb[:], rdb[:])
                nc.vector.tensor_tensor(out=rdb[:], in0=rdb[:], in1=aeff[:], op=Alu.mult)
                rdb_b = rdb[:].to_broadcast([128, MT, D])
                nc.gpsimd.tensor_tensor(out=nVv, in0=emVv, in1=onema_b, op=Alu.mult)
                nc.gpsimd.tensor_tensor(out=sqb[:].rearrange("q (t d) -> q t d", t=MT),
                                        in0=blVview, in1=rdb_b, op=Alu.mult)
                nc.gpsimd.tensor_tensor(out=nVt[:], in0=nVt[:], in1=sqb[:], op=Alu.add)
                nc.sync.dma_start(nV_d[p].rearrange("(t q) d -> q t d", q=128),
                                  nVt[:].rearrange("q (t d) -> q t d", t=MT))

                nS16 = sm.tile([128, MT], F32, tag="nS16")
                nc.vector.tensor_tensor(out=nS16[:], in0=P_S16[p][:], in1=aps16[:], op=Alu.add)
                nc.vector.tensor_scalar_min(nS16[:], nS16[:], S_MAX)
                nc.vector.tensor_scalar_max(nS16[:], nS16[:], 0.0)
                psD = poolX.tile([128, 130], F32, tag="tp")
                nc.tensor.matmul(psD[:128, 0:1], ones1x128[:], dec_ap, start=True, stop=True)
                dec128 = sm.tile([128, 1], F32, tag="dec128")
                nc.scalar.copy(dec128[:], psD[:128, 0:1])
                nc.vector.tensor_scalar_mul(nS16[:], nS16[:], dec128[:])
                colsum = sm.tile([128, 1], F32, tag="colsum")
                nc.vector.reduce_sum(colsum[:], nS16[:], axis=AX.X)
                psE = poolX.tile([128, 130], F32, tag="tp")
                nc.tensor.matmul(psE[:1, 0:1], colsum[:], ones128[:], start=True, stop=True)
                tot = sm.tile([1, 1], F32, tag="tot")
                nc.scalar.copy(tot[:], psE[:1, 0:1])
                nc.vector.tensor_scalar_add(tot[:], tot[:], EPS)
                nc.vector.reciprocal(tot[:], tot[:])
                nc.vector.tensor_scalar(tot[:], tot[:], BUDGET, 1.0, op0=Alu.mult, op1=Alu.min)
                psF = poolX.tile([128, 130], F32, tag="tp")
                nc.tensor.matmul(psF[:128, 0:1], ones1x128[:], tot[:], start=True, stop=True)
                sc128 = sm.tile([128, 1], F32, tag="sc128")
                nc.scalar.copy(sc128[:], psF[:128, 0:1])
                nc.vector.tensor_scalar_mul(nS16[:], nS16[:], sc128[:])
                nc.sync.dma_start(nS_d[p].rearrange("(t q) -> q t", q=128), nS16[:])

                nA16 = sm.tile([128, MT], F32, tag="nA16")
                nc.vector.tensor_scalar(nA16[:], aps16[:], -1.0, 1.0, op0=Alu.mult, op1=Alu.add)
                nc.vector.tensor_tensor(out=nA16[:], in0=nA16[:], in1=P_A16[p][:], op=Alu.mult)
                nc.sync.dma_start(nA_d[p].rearrange("(t q) -> q t", q=128), nA16[:])

    nc.compile()
    return nc


_CACHE = {}


def get_program(debug=False):
    key = bool(debug)
    if key not in _CACHE:
        _CACHE[key] = build_program(debug=debug)
    return _CACHE[key]


def shard_inputs(inputs):
    q = np.ascontiguousarray(np.asarray(inputs['q']).transpose(0, 2, 1, 3).reshape(PAIRS, N, D))
    qn = np.ascontiguousarray(np.asarray(inputs['q_nov']).transpose(0, 2, 1, 3).reshape(PAIRS, N, D))
    vn = np.ascontiguousarray(np.asarray(inputs['v_nov']).transpose(0, 2, 1, 3).reshape(PAIRS, N, D))
    sur = np.ascontiguousarray(np.asarray(inputs['surprise']).transpose(0, 2, 1).reshape(PAIRS, N))
    wn = np.ascontiguousarray(np.asarray(inputs['w_nov']).transpose(0, 2, 1).reshape(PAIRS, N))
    gtdw = np.ascontiguousarray(
        np.stack([np.asarray(inputs['g_em']), np.asarray(inputs['tau']),
                  np.asarray(inputs['decay']), np.asarray(inputs['ww'])], axis=-1
                 ).reshape(PAIRS, 4).astype(np.float32))
    emK = np.ascontiguousarray(np.asarray(inputs['em_K']).reshape(PAIRS, M, D))
    emV = np.ascontiguousarray(np.asarray(inputs['em_V']).reshape(PAIRS, M, D))
    emS = np.ascontiguousarray(np.asarray(inputs['em_S']).reshape(PAIRS, M))
    emA = np.ascontiguousarray(np.asarray(inputs['em_age']).reshape(PAIRS, M))
    in_maps = []
    for c in range(NCORES):
        s = slice(c * PPC, (c + 1) * PPC)
        in_maps.append({
            "q": q[s], "qn": qn[s], "vn": vn[s], "sur": sur[s], "wn": wn[s],
            "gtdw": gtdw[s], "emK": emK[s], "emV": emV[s], "emS": emS[s],
            "emA": emA[s],
        })
    return in_maps


def unshard_outputs(results):
    def cat(name):
        return np.concatenate([r[name] for r in results], axis=0)
    out = cat("out").reshape(BS, B, N, D).transpose(0, 2, 1, 3)
    nK = cat("nK").reshape(BS, B, M, D)
    nV = cat("nV").reshape(BS, B, M, D)
    nS = cat("nS").reshape(BS, B, M)
    nA = cat("nA").reshape(BS, B, M)
    return (np.ascontiguousarray(out), nK, nV, nS, nA)


def kernel(**inputs):
    assert int(inputs.get('C_cand', C)) == C
    nc = get_program(debug=False)
    in_maps = shard_inputs(inputs)
    res = run_bass_kernel_spmd(nc, in_maps, core_ids=list(range(NCORES)))
    return unshard_outputs(res.results)


# revision 33
# speedup vs baseline: 1.0539x; 1.0539x over previous
import sys

sys.path.insert(0, '/opt/trn_rl_repo')
import numpy as np
import concourse.bacc as bacc
import concourse.mybir as mybir
import concourse.tile as tile
from concourse.bass_utils import run_bass_kernel_spmd
from concourse.masks import make_identity

dt = mybir.dt
F32 = dt.float32
F32R = dt.float32r
Alu = mybir.AluOpType
Act = mybir.ActivationFunctionType
AX = mybir.AxisListType

BS, N, B, D, M = 4, 1024, 8, 64, 2048
NT, MT = N // 128, M // 128          # 8 n-tiles, 16 m-tiles
NQ = 4                               # n-quarters (2 n-tiles each)
PAIRS = BS * B
NCORES = 8
PPC = PAIRS // NCORES
C = 64
S_MAX = 4.0
BUDGET = 512.0
NEG = -1.0e9
EPS = 1e-8


def build_program(debug=False):
    nc = bacc.Bacc("TRN2", target_bir_lowering=False, debug=False)

    q_d = nc.dram_tensor("q", [PPC, N, D], F32, kind="ExternalInput").ap()
    qn_d = nc.dram_tensor("qn", [PPC, N, D], F32, kind="ExternalInput").ap()
    vn_d = nc.dram_tensor("vn", [PPC, N, D], F32, kind="ExternalInput").ap()
    sur_d = nc.dram_tensor("sur", [PPC, N], F32, kind="ExternalInput").ap()
    wn_d = nc.dram_tensor("wn", [PPC, N], F32, kind="ExternalInput").ap()
    gtdw_d = nc.dram_tensor("gtdw", [PPC, 4], F32, kind="ExternalInput").ap()
    emK_d = nc.dram_tensor("emK", [PPC, M, D], F32, kind="ExternalInput").ap()
    emV_d = nc.dram_tensor("emV", [PPC, M, D], F32, kind="ExternalInput").ap()
    emS_d = nc.dram_tensor("emS", [PPC, M], F32, kind="ExternalInput").ap()
    emA_d = nc.dram_tensor("emA", [PPC, M], F32, kind="ExternalInput").ap()

    out_d = nc.dram_tensor("out", [PPC, N, D], F32, kind="ExternalOutput").ap()
    nK_d = nc.dram_tensor("nK", [PPC, M, D], F32, kind="ExternalOutput").ap()
    nV_d = nc.dram_tensor("nV", [PPC, M, D], F32, kind="ExternalOutput").ap()
    nS_d = nc.dram_tensor("nS", [PPC, M], F32, kind="ExternalOutput").ap()
    nA_d = nc.dram_tensor("nA", [PPC, M], F32, kind="ExternalOutput").ap()
    if debug:
        dbg_v16 = nc.dram_tensor("dbg_v16", [PPC, N, 16], F32, kind="ExternalOutput").ap()
        dbg_nov = nc.dram_tensor("dbg_nov", [PPC, N], F32, kind="ExternalOutput").ap()
        dbg_cand = nc.dram_tensor("dbg_cand", [PPC, C], F32, kind="ExternalOutput").ap()
        dbg_ckv = nc.dram_tensor("dbg_ckv", [PPC, C, 128], F32, kind="ExternalOutput").ap()
        dbg_alpha = nc.dram_tensor("dbg_alpha", [PPC, C, M], F32, kind="ExternalOutput").ap()

    with tile.TileContext(nc) as tc:
        with (
            tc.tile_pool(name="const", bufs=1) as cpool,
            tc.tile_pool(name="persist", bufs=PPC) as pp,
            tc.tile_pool(name="rt1", bufs=1) as rt1,
            tc.tile_pool(name="rtA", bufs=2) as rtA,
            tc.tile_pool(name="rtW", bufs=2) as rtW,
            tc.tile_pool(name="rtK", bufs=2) as rtK,
            tc.tile_pool(name="rtB2", bufs=2) as rtB2,
            tc.tile_pool(name="rtC", bufs=3) as rtC,
            tc.tile_pool(name="sm", bufs=3) as sm,
            tc.tile_pool(name="novTp", bufs=2) as novT_pool,
            tc.tile_pool(name="psS", bufs=2, space="PSUM") as poolS,
            tc.tile_pool(name="psM", bufs=2, space="PSUM") as poolM,
            tc.tile_pool(name="psO", bufs=1, space="PSUM") as poolO,
            tc.tile_pool(name="psX", bufs=3, space="PSUM") as poolX,
        ):
            ident = cpool.tile([128, 128], F32)
            make_identity(nc, ident[:])
            ones128 = cpool.tile([128, 1], F32)
            nc.vector.memset(ones128[:], 1.0)
            ones1x128 = cpool.tile([1, 128], F32)
            nc.vector.memset(ones1x128[:], 1.0)

            # ---------- per-pair persistent tiles ----------
            P_emK, P_qn, P_vn = [], [], []
            P_sur, P_wn, P_sim, P_nov = [], [], [], []
            P_S16, P_A16, P_gtdw = [], [], []
            for p in range(PPC):
                emK_nat = pp.tile([128, MT * D], F32, tag="emK")
                nc.sync.dma_start(emK_nat[:].rearrange("q (t d) -> q t d", t=MT),
                                  emK_d[p].rearrange("(t q) d -> q t d", q=128))
                qn_nat = pp.tile([128, NT * D], F32, tag="qn")
                nc.sync.dma_start(qn_nat[:].rearrange("q (t d) -> q t d", t=NT),
                                  qn_d[p].rearrange("(t q) d -> q t d", q=128))
                vn_nat = pp.tile([128, NT * D], F32, tag="vn")
                nc.sync.dma_start(vn_nat[:].rearrange("q (t d) -> q t d", t=NT),
                                  vn_d[p].rearrange("(t q) d -> q t d", q=128))
                sur_t = pp.tile([128, NT], F32, tag="sur")
                nc.sync.dma_start(sur_t[:], sur_d[p].rearrange("(t q) -> q t", q=128))
                wn_t = pp.tile([128, NT], F32, tag="wn")
                nc.sync.dma_start(wn_t[:], wn_d[p].rearrange("(t q) -> q t", q=128))
                S16 = pp.tile([128, MT], F32, tag="S16")
                nc.sync.dma_start(S16[:], emS_d[p].rearrange("(t q) -> q t", q=128))
                A16 = pp.tile([128, MT], F32, tag="A16")
                nc.sync.dma_start(A16[:], emA_d[p].rearrange("(t q) -> q t", q=128))
                gt = pp.tile([1, 4], F32, tag="gtdw")
                nc.sync.dma_start(gt[:], gtdw_d[p].rearrange("(a c) -> a c", a=1))
                simmax = pp.tile([128, NT], F32, tag="simmax")
                nov_sb = pp.tile([128, NT], F32, tag="nov")
                P_emK.append(emK_nat)
                P_qn.append(qn_nat); P_vn.append(vn_nat)
                P_sur.append(sur_t); P_wn.append(wn_t)
                P_sim.append(simmax); P_nov.append(nov_sb)
                P_S16.append(S16); P_A16.append(A16); P_gtdw.append(gt)

            cand = cpool.tile([PPC, C], F32)
            csum = cpool.tile([PPC, 1], F32)

            # =================== READ PHASE ===================
            for p in range(PPC):
                q_nat = rt1.tile([128, NT * D], F32, tag="q_nat")
                nc.sync.dma_start(q_nat[:].rearrange("q (t d) -> q t d", t=NT),
                                  q_d[p].rearrange("(t q) d -> q t d", q=128))
                qT = rt1.tile([64, N], F32, tag="qT")
                qnT = rt1.tile([64, N], F32, tag="qnT")
                for t in range(NT):
                    psT = poolX.tile([128, 130], F32, tag="tp")
                    nc.tensor.transpose(psT[:64, :128], q_nat[:, t * D:(t + 1) * D], ident[:])
                    nc.scalar.copy(qT[:, t * 128:(t + 1) * 128], psT[:64, :128])
                    psT2 = poolX.tile([128, 130], F32, tag="tp")
                    nc.tensor.transpose(psT2[:64, :128], P_qn[p][:, t * D:(t + 1) * D], ident[:])
                    nc.scalar.copy(qnT[:, t * 128:(t + 1) * 128], psT2[:64, :128])

                KTs = rt1.tile([64, M], F32, tag="KTs")
                for t in range(MT):
                    psT = poolX.tile([128, 130], F32, tag="tp")
                    nc.tensor.transpose(psT[:64, :128], P_emK[p][:, t * D:(t + 1) * D], ident[:])
                    nc.scalar.copy(KTs[:, t * 128:(t + 1) * 128], psT[:64, :128])

                # V65r: [128, MT*65] fp32r: emV blocks + ones col
                V65f = rt1.tile([128, MT * 65], F32, tag="V65f")
                nc.sync.dma_start(
                    V65f[:].rearrange("q (t d) -> q t d", t=MT)[:, :, 0:D],
                    emV_d[p].rearrange("(t q) d -> q t d", q=128))
                nc.vector.memset(V65f[:].rearrange("q (t d) -> q t d", t=MT)[:, :, D:65], 1.0)
                V65r = rt1.tile([128, MT * 65], F32R, tag="V65r")
                nc.scalar.copy(V65r[:], V65f[:])

                def flush_tile(j, e_tile, wT_cur):
                    nq = j // 2
                    ii = j % 2
                    for t in range(MT):
                        psT = poolX.tile([128, 130], F32, tag="tp")
                        nc.tensor.transpose(psT[:128, :128],
                                            e_tile[:, t * 128:(t + 1) * 128], ident[:])
                        nc.scalar.copy(
                            wT_cur[:, t * 256 + ii * 128: t * 256 + (ii + 1) * 128],
                            psT[:128, :128])
                    if ii == 0:
                        return
                    psO = poolO.tile([65, 256], F32, tag="outmm")
                    for t in range(MT):
                        nc.tensor.matmul(
                            psO[:], V65r[:, t * 65:(t + 1) * 65],
                            wT_cur[:, t * 256:(t + 1) * 256],
                            start=(t == 0), stop=(t == MT - 1))
                    outT = sm.tile([65, 256], F32, tag="outT")
                    nc.scalar.copy(outT[:], psO[:])
                    outF = sm.tile([128, 130], F32, tag="outF")
                    for jj in range(2):
                        psT = poolX.tile([128, 130], F32, tag="tp")
                        nc.tensor.transpose(psT[:128, :65],
                                            outT[:, jj * 128:(jj + 1) * 128], ident[:65, :65])
                        nc.scalar.copy(outF[:, jj * 65:(jj + 1) * 65], psT[:128, :65])
                    den2 = sm.tile([128, 2], F32, tag="den2")
                    nc.vector.tensor_copy(
                        den2[:], outF[:].rearrange("q (t x) -> q t x", t=2)[:, :, 64:65])
                    rec2 = sm.tile([128, 2], F32, tag="rec2")
                    nc.vector.reciprocal(rec2[:], den2[:])
                    outfin = sm.tile([128, 2 * D], F32, tag="outfin")
                    for jj in range(2):
                        nc.vector.tensor_scalar_mul(
                            outfin[:, jj * D:(jj + 1) * D],
                            outF[:, jj * 65: jj * 65 + 64], rec2[:, jj:jj + 1])
                    nc.sync.dma_start(
                        out_d[p].rearrange("(t q) d -> q t d", q=128)[:, nq * 2:(nq + 1) * 2, :],
                        outfin[:].rearrange("q (t d) -> q t d", t=2))

                pend = None
                wT_cur = None
                for i in range(NT):
                    if i % 2 == 0:
                        wT_next = rtW.tile([128, MT * 256], F32R, tag="wT")
                    s_sb = rtA.tile([128, M], F32, tag="bigA")
                    for h in range(4):
                        ps = poolS.tile([128, 512], F32, tag="mmS")
                        nc.tensor.matmul(
                            ps[:], qT[:, i * 128:(i + 1) * 128],
                            KTs[:, h * 512:(h + 1) * 512], start=True, stop=True)
                        nc.scalar.copy(s_sb[:, h * 512:(h + 1) * 512], ps[:])
                    smax4 = sm.tile([128, 4], F32, tag="smax4")
                    for h in range(4):
                        ps = poolM.tile([128, 512], F32, tag="mmM")
                        nc.tensor.matmul(
                            ps[:], qnT[:, i * 128:(i + 1) * 128],
                            KTs[:, h * 512:(h + 1) * 512], start=True, stop=True)
                        nc.vector.reduce_max(smax4[:, h:h + 1], ps[:], axis=AX.X)
                    nc.vector.reduce_max(P_sim[p][:, i:i + 1], smax4[:], axis=AX.X)

                    v18 = sm.tile([128, 8], F32, tag="v18")
                    nc.vector.max(out=v18[:], in_=s_sb[:])
                    s2 = rtC.tile([128, M], F32, tag="bigC")
                    nc.vector.match_replace(out=s2[:], in_to_replace=v18[:],
                                            in_values=s_sb[:], imm_value=NEG)
                    v916 = sm.tile([128, 8], F32, tag="v916")
                    nc.vector.max(out=v916[:], in_=s2[:])
                    if debug:
                        v16c = sm.tile([128, 16], F32, tag="v16c")
                        nc.vector.tensor_copy(v16c[:, 0:8], v18[:])
                        nc.vector.tensor_copy(v16c[:, 8:16], v916[:])
                        nc.sync.dma_start(
                            dbg_v16[p].rearrange("(t q) k -> q t k", q=128)[:, i:i + 1, :],
                            v16c[:])
                    tstar = v916[:, 7:8]
                    tneg = sm.tile([128, 1], F32, tag="tneg")
                    nc.vector.tensor_scalar_mul(tneg[:], v18[:, 0:1], -1.0)
                    e_sb = rtC.tile([128, M], F32, tag="bigC")
                    nc.scalar.activation(e_sb[:], s_sb[:], Act.Exp, bias=tneg[:], scale=1.0)
                    nc.vector.tensor_scalar(
                        s_sb[:], s_sb[:], tstar, None, op0=Alu.is_ge)
                    nc.gpsimd.tensor_tensor(
                        out=e_sb[:], in0=e_sb[:], in1=s_sb[:], op=Alu.mult)
                    if pend is not None:
                        flush_tile(pend[0], pend[1], pend[2])
                    pend = (i, e_sb, wT_next)
                    wT_cur = wT_next
                flush_tile(pend[0], pend[1], pend[2])

                # novelty for this pair
                ms = sm.tile([128, NT], F32, tag="ms")
                nc.vector.tensor_scalar_max(ms[:], P_sim[p][:], 0.0)
                om = sm.tile([128, NT], F32, tag="om")
                nc.vector.tensor_scalar(om[:], ms[:], -1.0, 1.0, op0=Alu.mult, op1=Alu.add)
                ow = sm.tile([128, NT], F32, tag="ow")
                nc.vector.tensor_scalar(ow[:], P_wn[p][:], -1.0, 1.0, op0=Alu.mult, op1=Alu.add)
                nc.vector.tensor_tensor(out=om[:], in0=om[:], in1=ow[:], op=Alu.mult)
                nc.vector.tensor_tensor(out=ow[:], in0=P_wn[p][:], in1=P_sur[p][:], op=Alu.mult)
                nc.vector.tensor_tensor(out=P_nov[p][:], in0=om[:], in1=ow[:], op=Alu.add)
                if debug:
                    nc.sync.dma_start(dbg_nov[p].rearrange("(t q) -> q t", q=128), P_nov[p][:])

            # =================== TOP-64 BATCH ===================
            novT = novT_pool.tile([PPC, N], F32, tag="novT")
            for p in range(PPC):
                for t in range(NT):
                    nc.sync.dma_start(
                        novT[p:p + 1, t * 128:(t + 1) * 128],
                        P_nov[p][:, t:t + 1])
            cur = novT
            for r in range(C // 8):
                nc.vector.max(out=cand[:, r * 8:(r + 1) * 8], in_=cur[:])
                if r < C // 8 - 1:
                    nxt = novT_pool.tile([PPC, N], F32, tag="novT")
                    nc.vector.match_replace(out=nxt[:], in_to_replace=cand[:, r * 8:(r + 1) * 8],
                                            in_values=cur[:], imm_value=NEG)
                    cur = nxt
            nc.vector.reduce_sum(csum[:], cand[:], axis=AX.X)
            if debug:
                nc.sync.dma_start(dbg_cand, cand[:])

            # =================== WRITE PHASE ===================
            for p in range(PPC):
                g_ap = P_gtdw[p][0:1, 0:1]
                tau_ap = P_gtdw[p][0:1, 1:2]
                dec_ap = P_gtdw[p][0:1, 2:3]
                ww_ap = P_gtdw[p][0:1, 3:4]

                candp0 = sm.tile([1, C], F32, tag="candp0")
                nc.sync.dma_start(candp0[:], cand[p:p + 1, :])
                csump0 = sm.tile([1, 1], F32, tag="csump0")
                nc.sync.dma_start(csump0[:], csum[p:p + 1, :])
                psB = poolX.tile([128, 130], F32, tag="tp")
                nc.tensor.matmul(psB[:128, 0:C], ones1x128[:], candp0[:],
                                 start=True, stop=True)
                candB = sm.tile([128, C], F32, tag="candB")
                nc.scalar.copy(candB[:], psB[:128, 0:C])
                psC1 = poolX.tile([128, 130], F32, tag="tp")
                nc.tensor.matmul(psC1[:C, 0:1], candp0[:], ones1x128[:, 0:1],
                                 start=True, stop=True)
                candcol = sm.tile([C, 1], F32, tag="candcol")
                nc.scalar.copy(candcol[:], psC1[:C, 0:1])

                ohT = rt1.tile([128, NT * C], F32, tag="q_nat")
                for i in range(NT):
                    nc.vector.tensor_scalar(
                        ohT[:, i * C:(i + 1) * C], candB[:], P_nov[p][:, i:i + 1], None,
                        op0=Alu.is_equal)
                psC = poolX.tile([128, 130], F32, tag="tp")
                for i in range(NT):
                    nc.tensor.matmul(psC[:C, 0:D], ohT[:, i * C:(i + 1) * C],
                                     P_qn[p][:, i * D:(i + 1) * D],
                                     start=(i == 0), stop=(i == NT - 1),
                                     skip_group_check=True)
                for i in range(NT):
                    nc.tensor.matmul(psC[:C, D:2 * D], ohT[:, i * C:(i + 1) * C],
                                     P_vn[p][:, i * D:(i + 1) * D],
                                     start=(i == 0), stop=(i == NT - 1),
                                     skip_group_check=True)
                ckv = sm.tile([C, 2 * D], F32, tag="ckv")
                nc.scalar.copy(ckv[:], psC[:C, 0:2 * D])
                if debug:
                    nc.sync.dma_start(dbg_ckv[p], ckv[:])

                sq = sm.tile([C, D], F32, tag="sqck")
                nrm2 = sm.tile([C, 1], F32, tag="nrm2")
                nc.vector.scalar_tensor_tensor(
                    out=sq[:], in0=ckv[:, 0:D], scalar=1.0, in1=ckv[:, 0:D],
                    op0=Alu.mult, op1=Alu.mult, accum_out=nrm2[:])
                rinv = sm.tile([C, 1], F32, tag="rinv")
                nc.vector.reciprocal(rinv[:], nrm2[:])
                rn = sm.tile([C, 1], F32, tag="rn")
                nc.scalar.activation(rn[:], rinv[:], Act.Sqrt)
                bl_rhs = sm.tile([C, 129], F32, tag="bl_rhs")
                nc.vector.tensor_scalar_mul(bl_rhs[:, 0:D], ckv[:, 0:D], rn[:])
                nc.scalar.copy(bl_rhs[:, D:2 * D], ckv[:, D:2 * D])
                nc.vector.memset(bl_rhs[:, 128:129], 1.0)

                ckT65 = sm.tile([65, C], F32, tag="ckT65")
                psT = poolX.tile([128, 130], F32, tag="tp")
                nc.tensor.transpose(psT[:D, 0:C], bl_rhs[:, 0:D], ident[:C, :C])
                nc.scalar.copy(ckT65[0:D, :], psT[:D, 0:C])
                negww = sm.tile([1, 1], F32, tag="negww")
                nc.vector.tensor_scalar_mul(negww[:], ww_ap, -1.0)
                psW = poolX.tile([128, 130], F32, tag="tp")
                nc.tensor.matmul(psW[:1, 0:C], negww[:], ones1x128[:, 0:C],
                                 start=True, stop=True)
                nc.scalar.copy(ckT65[64:65, :], psW[:1, 0:C])

                KTf = rtK.tile([65, M], F32, tag="KTf")
                for t in range(MT):
                    psT = poolX.tile([128, 130], F32, tag="tp")
                    nc.tensor.transpose(psT[:64, :128], P_emK[p][:, t * D:(t + 1) * D], ident[:])
                    nc.scalar.copy(KTf[0:64, t * 128:(t + 1) * 128], psT[:64, :128])
                sSrow = rt1.tile([1, M], F32, tag="sSrow")
                nc.sync.dma_start(sSrow[:], emS_d[p].rearrange("(a m) -> a m", a=1))
                nc.scalar.copy(KTf[64:65, :], sSrow[:])

                invtau1 = sm.tile([1, 1], F32, tag="invtau1")
                nc.vector.tensor_scalar_max(invtau1[:], tau_ap, 0.01)
                nc.vector.reciprocal(invtau1[:], invtau1[:])
                psI = poolX.tile([128, 130], F32, tag="tp")
                nc.tensor.matmul(psI[:C, 0:1], ones1x128[:, 0:C], invtau1[:],
                                 start=True, stop=True)
                invtau = sm.tile([C, 1], F32, tag="invtau")
                nc.scalar.copy(invtau[:], psI[:C, 0:1])

                slotraw = rtA.tile([C, M], F32, tag="bigA")
                for h in range(4):
                    psL = poolM.tile([128, 512], F32, tag="mmM")
                    nc.tensor.matmul(
                        psL[:C, :], ckT65[:], KTf[:, h * 512:(h + 1) * 512],
                        start=True, stop=True)
                    nc.scalar.copy(slotraw[:, h * 512:(h + 1) * 512], psL[:C, :])
                sw = rtC.tile([C, M], F32, tag="bigC")
                rmax = sm.tile([C, 1], F32, tag="rmax")
                nc.vector.reduce_max(rmax[:], slotraw[:], axis=AX.X)
                nbias = sm.tile([C, 1], F32, tag="nbias")
                nc.vector.tensor_tensor(out=nbias[:], in0=rmax[:], in1=invtau[:], op=Alu.mult)
                nc.vector.tensor_scalar_mul(nbias[:], nbias[:], -1.0)
                rsum = sm.tile([C, 1], F32, tag="rsum")
                nc.scalar.activation(sw[:], slotraw[:], Act.Exp, bias=nbias[:],
                                     scale=invtau[:], accum_out=rsum[:])

                gs1 = sm.tile([1, 1], F32, tag="gs1")
                nc.vector.tensor_scalar_add(gs1[:], csump0[:], EPS)
                nc.vector.reciprocal(gs1[:], gs1[:])
                nc.vector.tensor_tensor(out=gs1[:], in0=gs1[:], in1=g_ap, op=Alu.mult)
                psG = poolX.tile([128, 130], F32, tag="tp")
                nc.tensor.matmul(psG[:C, 0:1], ones1x128[:, 0:C], gs1[:], start=True, stop=True)
                gsC = sm.tile([C, 1], F32, tag="gsC")
                nc.scalar.copy(gsC[:], psG[:C, 0:1])
                alphacol = sm.tile([C, 1], F32, tag="alphacol")
                nc.vector.tensor_scalar_mul(alphacol[:], candcol[:], gsC[:])
                rr = sm.tile([C, 1], F32, tag="rr")
                nc.vector.reciprocal(rr[:], rsum[:])
                nc.vector.tensor_tensor(out=alphacol[:], in0=alphacol[:], in1=rr[:], op=Alu.mult)
                nc.vector.tensor_scalar_mul(sw[:], sw[:], alphacol[:])
                alpha = sw
                if debug:
                    nc.sync.dma_start(dbg_alpha[p], alpha[:])

                blKV = rtB2.tile([128, MT * 129], F32, tag="blKV")
                for t in range(MT):
                    psB2 = poolX.tile([128, 130], F32, tag="tp")
                    nc.tensor.matmul(psB2[:128, 0:129], alpha[:, t * 128:(t + 1) * 128],
                                     bl_rhs[:], start=True, stop=True)
                    nc.scalar.copy(blKV[:, t * 129:(t + 1) * 129], psB2[:128, 0:129])

                aps16 = sm.tile([128, MT], F32, tag="aps16")
                nc.vector.tensor_copy(
                    aps16[:], blKV[:].rearrange("q (t x) -> q t x", t=MT)[:, :, 128:129])
                masku = sm.tile([128, MT], F32, tag="masku")
                nc.vector.tensor_scalar(masku[:], aps16[:], EPS, None, op0=Alu.is_gt)
                blKview = blKV[:].rearrange("q (t x) -> q t x", t=MT)[:, :, 0:D]
                blVview = blKV[:].rearrange("q (t x) -> q t x", t=MT)[:, :, D:2 * D]
                sqb = rt1.tile([128, MT * D], F32, tag="sqb")
                nc.gpsimd.tensor_tensor(out=sqb[:].rearrange("q (t d) -> q t d", t=MT),
                                        in0=blKview, in1=blKview, op=Alu.mult)
                nrm2b = sm.tile([128, MT], F32, tag="nrm2b")
                nc.vector.reduce_sum(nrm2b[:], sqb[:].rearrange("q (t d) -> q t d", t=MT),
                                     axis=AX.X)
                nc.vector.tensor_scalar_max(nrm2b[:], nrm2b[:], 1e-30)
                rnb = sm.tile([128, MT], F32, tag="rnb")
                nc.vector.reciprocal(rnb[:], nrm2b[:])
                nc.scalar.activation(rnb[:], rnb[:], Act.Sqrt)
                nc.vector.tensor_tensor(out=rnb[:], in0=rnb[:], in1=masku[:], op=Alu.mult)
                aeff = sm.tile([128, MT], F32, tag="aeff")
                nc.vector.tensor_scalar_min(aeff[:], aps16[:], 1.0)
                nc.vector.tensor_tensor(out=aeff[:], in0=aeff[:], in1=masku[:], op=Alu.mult)
                onema = sm.tile([128, MT], F32, tag="onema")
                nc.vector.tensor_scalar(onema[:], aeff[:], -1.0, 1.0, op0=Alu.mult, op1=Alu.add)
                scalK = sm.tile([128, MT], F32, tag="scalK")
                nc.vector.tensor_tensor(out=scalK[:], in0=aeff[:], in1=rnb[:], op=Alu.mult)

                nKt = rtA.tile([128, MT * D], F32, tag="bigA")
                nVt = rtC.tile([128, MT * D], F32, tag="bigC")
                onema_b = onema[:].to_broadcast([128, MT, D])
                scalK_b = scalK[:].to_broadcast([128, MT, D])
                nKv = nKt[:].rearrange("q (t d) -> q t d", t=MT)
                nVv = nVt[:].rearrange("q (t d) -> q t d", t=MT)
                emKv = P_emK[p][:].rearrange("q (t d) -> q t d", t=MT)
                emVw = rt1.tile([128, MT * D], F32, tag="emVw")
                nc.sync.dma_start(emVw[:].rearrange("q (t d) -> q t d", t=MT),
                                  emV_d[p].rearrange("(t q) d -> q t d", q=128))
                emVv = emVw[:].rearrange("q (t d) -> q t d", t=MT)
                nc.gpsimd.tensor_tensor(out=nKv, in0=emKv, in1=onema_b, op=Alu.mult)
                sqb2 = rtC.tile([128, MT * D], F32, tag="bigC")
                nc.gpsimd.tensor_tensor(out=sqb2[:].rearrange("q (t d) -> q t d", t=MT),
                                        in0=blKview, in1=scalK_b, op=Alu.mult)
                nc.gpsimd.tensor_tensor(out=nKt[:], in0=nKt[:], in1=sqb2[:], op=Alu.add)
                nc.sync.dma_start(nK_d[p].rearrange("(t q) d -> q t d", q=128),
                                  nKt[:].rearrange("q (t d) -> q t d", t=MT))
                # new_V: scale = aeff / max(aps, eps)
                rdb = sm.tile([128, MT], F32, tag="rdb")
                nc.vector.tensor_scalar_max(rdb[:], aps16[:], EPS)
                nc.vector.reciprocal(rdb[:], rdb[:])
                nc.vector.tensor_tensor(out=rdb[:], in0=rdb[:], in1=aeff[:], op=Alu.mult)
                rdb_b = rdb[:].to_broadcast([128, MT, D])
                nc.gpsimd.tensor_tensor(out=nVv, in0=emVv, in1=onema_b, op=Alu.mult)
                nc.gpsimd.tensor_tensor(out=sqb[:].rearrange("q (t d) -> q t d", t=MT),
                                        in0=blVview, in1=rdb_b, op=Alu.mult)
                nc.gpsimd.tensor_tensor(out=nVt[:], in0=nVt[:], in1=sqb[:], op=Alu.add)
                nc.sync.dma_start(nV_d[p].rearrange("(t q) d -> q t d", q=128),
                                  nVt[:].rearrange("q (t d) -> q t d", t=MT))

                nS16 = sm.tile([128, MT], F32, tag="nS16")
                nc.vector.tensor_tensor(out=nS16[:], in0=P_S16[p][:], in1=aps16[:], op=Alu.add)
                nc.vector.tensor_scalar_min(nS16[:], nS16[:], S_MAX)
                nc.vector.tensor_scalar_max(nS16[:], nS16[:], 0.0)
                psD = poolX.tile([128, 130], F32, tag="tp")
                nc.tensor.matmul(psD[:128, 0:1], ones1x128[:], dec_ap, start=True, stop=True)
                dec128 = sm.tile([128, 1], F32, tag="dec128")
                nc.scalar.copy(dec128[:], psD[:128, 0:1])
                nc.vector.tensor_scalar_mul(nS16[:], nS16[:], dec128[:])
                colsum = sm.tile([128, 1], F32, tag="colsum")
                nc.vector.reduce_sum(colsum[:], nS16[:], axis=AX.X)
                psE = poolX.tile([128, 130], F32, tag="tp")
                nc.tensor.matmul(psE[:1, 0:1], colsum[:], ones128[:], start=True, stop=True)
                tot = sm.tile([1, 1], F32, tag="tot")
                nc.scalar.copy(tot[:], psE[:1, 0:1])
                nc.vector.tensor_scalar_add(tot[:], tot[:], EPS)
                nc.vector.reciprocal(tot[:], tot[:])
                nc.vector.tensor_scalar(tot[:], tot[:], BUDGET, 1.0, op0=Alu.mult, op1=Alu.min)
                psF = poolX.tile([128, 130], F32, tag="tp")
                nc.tensor.matmul(psF[:128, 0:1], ones1x128[:], tot[:], start=True, stop=True)
                sc128 = sm.tile([128, 1], F32, tag="sc128")
                nc.scalar.copy(sc128[:], psF[:128, 0:1])
                nc.vector.tensor_scalar_mul(nS16[:], nS16[:], sc128[:])
                nc.sync.dma_start(nS_d[p].rearrange("(t q) -> q t", q=128), nS16[:])

                nA16 = sm.tile([128, MT], F32, tag="nA16")
                nc.vector.tensor_scalar(nA16[:], aps16[:], -1.0, 1.0, op0=Alu.mult, op1=Alu.add)
                nc.vector.tensor_tensor(out=nA16[:], in0=nA16[:], in1=P_A16[p][:], op=Alu.mult)
                nc.sync.dma_start(nA_d[p].rearrange("(t q) -> q t", q=128), nA16[:])

    nc.compile()
    return nc


_CACHE = {}


def get_program(debug=False):
    key = bool(debug)
    if key not in _CACHE:
        _CACHE[key] = build_program(debug=debug)
    return _CACHE[key]


def shard_inputs(inputs):
    q = np.ascontiguousarray(np.asarray(inputs['q']).transpose(0, 2, 1, 3).reshape(PAIRS, N, D))
    qn = np.ascontiguousarray(np.asarray(inputs['q_nov']).transpose(0, 2, 1, 3).reshape(PAIRS, N, D))
    vn = np.ascontiguousarray(np.asarray(inputs['v_nov']).transpose(0, 2, 1, 3).reshape(PAIRS, N, D))
    sur = np.ascontiguousarray(np.asarray(inputs['surprise']).transpose(0, 2, 1).reshape(PAIRS, N))
    wn = np.ascontiguousarray(np.asarray(inputs['w_nov']).transpose(0, 2, 1).reshape(PAIRS, N))
    gtdw = np.ascontiguousarray(
        np.stack([np.asarray(inputs['g_em']), np.asarray(inputs['tau']),
                  np.asarray(inputs['decay']), np.asarray(inputs['ww'])], axis=-1
                 ).reshape(PAIRS, 4).astype(np.float32))
    emK = np.ascontiguousarray(np.asarray(inputs['em_K']).reshape(PAIRS, M, D))
    emV = np.ascontiguousarray(np.asarray(inputs['em_V']).reshape(PAIRS, M, D))
    emS = np.ascontiguousarray(np.asarray(inputs['em_S']).reshape(PAIRS, M))
    emA = np.ascontiguousarray(np.asarray(inputs['em_age']).reshape(PAIRS, M))
    in_maps = []
    for c in range(NCORES):
        s = slice(c * PPC, (c + 1) * PPC)
        in_maps.append({
            "q": q[s], "qn": qn[s], "vn": vn[s], "sur": sur[s], "wn": wn[s],
            "gtdw": gtdw[s], "emK": emK[s], "emV": emV[s], "emS": emS[s],
            "emA": emA[s],
        })
    return in_maps


def unshard_outputs(results):
    def cat(name):
        return np.concatenate([r[name] for r in results], axis=0)
    out = cat("out").reshape(BS, B, N, D).transpose(0, 2, 1, 3)
    nK = cat("nK").reshape(BS, B, M, D)
    nV = cat("nV").reshape(BS, B, M, D)
    nS = cat("nS").reshape(BS, B, M)
    nA = cat("nA").reshape(BS, B, M)
    return (np.ascontiguousarray(out), nK, nV, nS, nA)


def kernel(**inputs):
    assert int(inputs.get('C_cand', C)) == C
    nc = get_program(debug=False)
    in_maps = shard_inputs(inputs)
    res = run_bass_kernel_spmd(nc, in_maps, core_ids=list(range(NCORES)))
    return unshard_outputs(res.results)


# revision 39
# speedup vs baseline: 1.0744x; 1.0195x over previous
import sys

sys.path.insert(0, '/opt/trn_rl_repo')
import numpy as np
import concourse.bacc as bacc
import concourse.mybir as mybir
import concourse.tile as tile
from concourse.bass_utils import run_bass_kernel_spmd
from concourse.masks import make_identity

dt = mybir.dt
F32 = dt.float32
F32R = dt.float32r
Alu = mybir.AluOpType
Act = mybir.ActivationFunctionType
AX = mybir.AxisListType

BS, N, B, D, M = 4, 1024, 8, 64, 2048
NT, MT = N // 128, M // 128          # 8 n-tiles, 16 m-tiles
NQ = 4                               # n-quarters (2 n-tiles each)
PAIRS = BS * B
NCORES = 8
PPC = PAIRS // NCORES
C = 64
S_MAX = 4.0
BUDGET = 512.0
NEG = -1.0e9
EPS = 1e-8


def build_program(debug=False):
    nc = bacc.Bacc("TRN2", target_bir_lowering=False, debug=False)

    q_d = nc.dram_tensor("q", [PPC, N, D], F32, kind="ExternalInput").ap()
    qn_d = nc.dram_tensor("qn", [PPC, N, D], F32, kind="ExternalInput").ap()
    vn_d = nc.dram_tensor("vn", [PPC, N, D], F32, kind="ExternalInput").ap()
    sur_d = nc.dram_tensor("sur", [PPC, N], F32, kind="ExternalInput").ap()
    wn_d = nc.dram_tensor("wn", [PPC, N], F32, kind="ExternalInput").ap()
    gtdw_d = nc.dram_tensor("gtdw", [PPC, 4], F32, kind="ExternalInput").ap()
    emK_d = nc.dram_tensor("emK", [PPC, M, D], F32, kind="ExternalInput").ap()
    emV_d = nc.dram_tensor("emV", [PPC, M, D], F32, kind="ExternalInput").ap()
    emS_d = nc.dram_tensor("emS", [PPC, M], F32, kind="ExternalInput").ap()
    emA_d = nc.dram_tensor("emA", [PPC, M], F32, kind="ExternalInput").ap()

    out_d = nc.dram_tensor("out", [PPC, N, D], F32, kind="ExternalOutput").ap()
    nK_d = nc.dram_tensor("nK", [PPC, M, D], F32, kind="ExternalOutput").ap()
    nV_d = nc.dram_tensor("nV", [PPC, M, D], F32, kind="ExternalOutput").ap()
    nS_d = nc.dram_tensor("nS", [PPC, M], F32, kind="ExternalOutput").ap()
    nA_d = nc.dram_tensor("nA", [PPC, M], F32, kind="ExternalOutput").ap()
    if debug:
        dbg_v16 = nc.dram_tensor("dbg_v16", [PPC, N, 16], F32, kind="ExternalOutput").ap()
        dbg_nov = nc.dram_tensor("dbg_nov", [PPC, N], F32, kind="ExternalOutput").ap()
        dbg_cand = nc.dram_tensor("dbg_cand", [PPC, C], F32, kind="ExternalOutput").ap()
        dbg_ckv = nc.dram_tensor("dbg_ckv", [PPC, C, 128], F32, kind="ExternalOutput").ap()
        dbg_alpha = nc.dram_tensor("dbg_alpha", [PPC, C, M], F32, kind="ExternalOutput").ap()

    with tile.TileContext(nc) as tc:
        with (
            tc.tile_pool(name="const", bufs=1) as cpool,
            tc.tile_pool(name="persist", bufs=PPC) as pp,
            tc.tile_pool(name="rt1", bufs=1) as rt1,
            tc.tile_pool(name="rtA", bufs=4) as rtA,
            tc.tile_pool(name="rtW", bufs=1) as rtW,
            tc.tile_pool(name="rtK", bufs=2) as rtK,
            tc.tile_pool(name="rtB2", bufs=2) as rtB2,
            tc.tile_pool(name="rtC", bufs=3) as rtC,
            tc.tile_pool(name="sm", bufs=3) as sm,
            tc.tile_pool(name="hot", bufs=5) as hot,
            tc.tile_pool(name="novTp", bufs=2) as novT_pool,
            tc.tile_pool(name="psS", bufs=2, space="PSUM") as poolS,
            tc.tile_pool(name="psM", bufs=2, space="PSUM") as poolM,
            tc.tile_pool(name="psO", bufs=1, space="PSUM") as poolO,
            tc.tile_pool(name="psX", bufs=3, space="PSUM") as poolX,
        ):
            ident = cpool.tile([128, 128], F32)
            make_identity(nc, ident[:])
            ones128 = cpool.tile([128, 1], F32)
            nc.vector.memset(ones128[:], 1.0)
            ones1x128 = cpool.tile([1, 128], F32)
            nc.vector.memset(ones1x128[:], 1.0)

            # ---------- per-pair persistent tiles ----------
            P_emK, P_qn, P_vn = [], [], []
            P_sur, P_wn, P_sim, P_nov = [], [], [], []
            P_S16, P_A16, P_gtdw = [], [], []
            for p in range(PPC):
                emK_nat = pp.tile([128, MT * D], F32, tag="emK")
                nc.sync.dma_start(emK_nat[:].rearrange("q (t d) -> q t d", t=MT),
                                  emK_d[p].rearrange("(t q) d -> q t d", q=128))
                qn_nat = pp.tile([128, NT * D], F32, tag="qn")
                nc.sync.dma_start(qn_nat[:].rearrange("q (t d) -> q t d", t=NT),
                                  qn_d[p].rearrange("(t q) d -> q t d", q=128))
                vn_nat = pp.tile([128, NT * D], F32, tag="vn")
                nc.sync.dma_start(vn_nat[:].rearrange("q (t d) -> q t d", t=NT),
                                  vn_d[p].rearrange("(t q) d -> q t d", q=128))
                sur_t = pp.tile([128, NT], F32, tag="sur")
                nc.sync.dma_start(sur_t[:], sur_d[p].rearrange("(t q) -> q t", q=128))
                wn_t = pp.tile([128, NT], F32, tag="wn")
                nc.sync.dma_start(wn_t[:], wn_d[p].rearrange("(t q) -> q t", q=128))
                S16 = pp.tile([128, MT], F32, tag="S16")
                nc.sync.dma_start(S16[:], emS_d[p].rearrange("(t q) -> q t", q=128))
                A16 = pp.tile([128, MT], F32, tag="A16")
                nc.sync.dma_start(A16[:], emA_d[p].rearrange("(t q) -> q t", q=128))
                gt = pp.tile([1, 4], F32, tag="gtdw")
                nc.sync.dma_start(gt[:], gtdw_d[p].rearrange("(a c) -> a c", a=1))
                simmax = pp.tile([128, NT], F32, tag="simmax")
                nov_sb = pp.tile([128, NT], F32, tag="nov")
                P_emK.append(emK_nat)
                P_qn.append(qn_nat); P_vn.append(vn_nat)
                P_sur.append(sur_t); P_wn.append(wn_t)
                P_sim.append(simmax); P_nov.append(nov_sb)
                P_S16.append(S16); P_A16.append(A16); P_gtdw.append(gt)

            cand = cpool.tile([PPC, C], F32)
            csum = cpool.tile([PPC, 1], F32)

            # =================== READ PHASE ===================
            for p in range(PPC):
                q_nat = rt1.tile([128, NT * D], F32, tag="q_nat")
                nc.sync.dma_start(q_nat[:].rearrange("q (t d) -> q t d", t=NT),
                                  q_d[p].rearrange("(t q) d -> q t d", q=128))
                qT = rt1.tile([64, N], F32, tag="qT")
                qnT = rt1.tile([64, N], F32, tag="qnT")
                for t in range(NT):
                    psT = poolX.tile([128, 130], F32, tag="tp")
                    nc.tensor.transpose(psT[:64, :128], q_nat[:, t * D:(t + 1) * D], ident[:])
                    nc.scalar.copy(qT[:, t * 128:(t + 1) * 128], psT[:64, :128])
                    psT2 = poolX.tile([128, 130], F32, tag="tp")
                    nc.tensor.transpose(psT2[:64, :128], P_qn[p][:, t * D:(t + 1) * D], ident[:])
                    nc.scalar.copy(qnT[:, t * 128:(t + 1) * 128], psT2[:64, :128])

                KTs = rt1.tile([64, M], F32, tag="KTs")
                for t in range(MT):
                    psT = poolX.tile([128, 130], F32, tag="tp")
                    nc.tensor.transpose(psT[:64, :128], P_emK[p][:, t * D:(t + 1) * D], ident[:])
                    nc.scalar.copy(KTs[:, t * 128:(t + 1) * 128], psT[:64, :128])

                # V65r: [128, MT*65] fp32r: emV blocks + ones col
                V65f = rt1.tile([128, MT * 65], F32, tag="V65f")
                nc.sync.dma_start(
                    V65f[:].rearrange("q (t d) -> q t d", t=MT)[:, :, 0:D],
                    emV_d[p].rearrange("(t q) d -> q t d", q=128))
                nc.vector.memset(V65f[:].rearrange("q (t d) -> q t d", t=MT)[:, :, D:65], 1.0)
                V65r = rt1.tile([128, MT * 65], F32R, tag="V65r")
                nc.scalar.copy(V65r[:], V65f[:])

                def flush_tile(j, e_tile, wT_cur):
                    nq = j // 2
                    ii = j % 2
                    for t in range(MT):
                        psT = poolX.tile([128, 130], F32, tag="tp")
                        nc.tensor.transpose(psT[:128, :128],
                                            e_tile[:, t * 128:(t + 1) * 128], ident[:])
                        nc.scalar.copy(
                            wT_cur[:, t * 256 + ii * 128: t * 256 + (ii + 1) * 128],
                            psT[:128, :128])
                    if ii == 0:
                        return
                    psO = poolO.tile([65, 256], F32, tag="outmm")
                    for t in range(MT):
                        nc.tensor.matmul(
                            psO[:], V65r[:, t * 65:(t + 1) * 65],
                            wT_cur[:, t * 256:(t + 1) * 256],
                            start=(t == 0), stop=(t == MT - 1))
                    outT = sm.tile([65, 256], F32, tag="outT")
                    nc.scalar.copy(outT[:], psO[:])
                    outF = sm.tile([128, 130], F32, tag="outF")
                    for jj in range(2):
                        psT = poolX.tile([128, 130], F32, tag="tp")
                        nc.tensor.transpose(psT[:128, :65],
                                            outT[:, jj * 128:(jj + 1) * 128], ident[:65, :65])
                        nc.scalar.copy(outF[:, jj * 65:(jj + 1) * 65], psT[:128, :65])
                    den2 = sm.tile([128, 2], F32, tag="den2")
                    nc.vector.tensor_copy(
                        den2[:], outF[:].rearrange("q (t x) -> q t x", t=2)[:, :, 64:65])
                    rec2 = sm.tile([128, 2], F32, tag="rec2")
                    nc.vector.reciprocal(rec2[:], den2[:])
                    outfin = sm.tile([128, 2 * D], F32, tag="outfin")
                    for jj in range(2):
                        nc.vector.tensor_scalar_mul(
                            outfin[:, jj * D:(jj + 1) * D],
                            outF[:, jj * 65: jj * 65 + 64], rec2[:, jj:jj + 1])
                    nc.sync.dma_start(
                        out_d[p].rearrange("(t q) d -> q t d", q=128)[:, nq * 2:(nq + 1) * 2, :],
                        outfin[:].rearrange("q (t d) -> q t d", t=2))

                pend = None
                wT_cur = None
                for i in range(NT):
                    if i % 2 == 0:
                        wT_next = rtW.tile([128, MT * 256], F32R, tag="wT")
                    s_sb = rtA.tile([128, M], F32, tag="bigA")
                    for h in range(4):
                        ps = poolS.tile([128, 512], F32, tag="mmS")
                        nc.tensor.matmul(
                            ps[:], qT[:, i * 128:(i + 1) * 128],
                            KTs[:, h * 512:(h + 1) * 512], start=True, stop=True)
                        nc.scalar.copy(s_sb[:, h * 512:(h + 1) * 512], ps[:])
                    smax4 = hot.tile([128, 4], F32, tag="smax4")
                    for h in range(4):
                        ps = poolM.tile([128, 512], F32, tag="mmM")
                        nc.tensor.matmul(
                            ps[:], qnT[:, i * 128:(i + 1) * 128],
                            KTs[:, h * 512:(h + 1) * 512], start=True, stop=True)
                        nc.vector.reduce_max(smax4[:, h:h + 1], ps[:], axis=AX.X)
                    nc.vector.reduce_max(P_sim[p][:, i:i + 1], smax4[:], axis=AX.X)

                    v18 = hot.tile([128, 8], F32, tag="v18")
                    nc.vector.max(out=v18[:], in_=s_sb[:])
                    s2 = rtC.tile([128, M], F32, tag="bigC")
                    nc.vector.match_replace(out=s2[:], in_to_replace=v18[:],
                                            in_values=s_sb[:], imm_value=NEG)
                    v916 = hot.tile([128, 8], F32, tag="v916")
                    nc.vector.max(out=v916[:], in_=s2[:])
                    if debug:
                        v16c = sm.tile([128, 16], F32, tag="v16c")
                        nc.vector.tensor_copy(v16c[:, 0:8], v18[:])
                        nc.vector.tensor_copy(v16c[:, 8:16], v916[:])
                        nc.sync.dma_start(
                            dbg_v16[p].rearrange("(t q) k -> q t k", q=128)[:, i:i + 1, :],
                            v16c[:])
                    tstar = v916[:, 7:8]
                    tneg = hot.tile([128, 1], F32, tag="tneg")
                    nc.vector.tensor_scalar_mul(tneg[:], v18[:, 0:1], -1.0)
                    e_sb = rtC.tile([128, M], F32, tag="bigC")
                    nc.scalar.activation(e_sb[:], s_sb[:], Act.Exp, bias=tneg[:], scale=1.0)
                    nc.vector.tensor_scalar(
                        s_sb[:], s_sb[:], tstar, None, op0=Alu.is_ge)
                    nc.gpsimd.tensor_tensor(
                        out=e_sb[:], in0=e_sb[:], in1=s_sb[:], op=Alu.mult)
                    if pend is not None:
                        flush_tile(pend[0], pend[1], pend[2])
                    pend = (i, e_sb, wT_next)
                    wT_cur = wT_next
                flush_tile(pend[0], pend[1], pend[2])

                # novelty for this pair
                ms = sm.tile([128, NT], F32, tag="ms")
                nc.vector.tensor_scalar_max(ms[:], P_sim[p][:], 0.0)
                om = sm.tile([128, NT], F32, tag="om")
                nc.vector.tensor_scalar(om[:], ms[:], -1.0, 1.0, op0=Alu.mult, op1=Alu.add)
                ow = sm.tile([128, NT], F32, tag="ow")
                nc.vector.tensor_scalar(ow[:], P_wn[p][:], -1.0, 1.0, op0=Alu.mult, op1=Alu.add)
                nc.vector.tensor_tensor(out=om[:], in0=om[:], in1=ow[:], op=Alu.mult)
                nc.vector.tensor_tensor(out=ow[:], in0=P_wn[p][:], in1=P_sur[p][:], op=Alu.mult)
                nc.vector.tensor_tensor(out=P_nov[p][:], in0=om[:], in1=ow[:], op=Alu.add)
                if debug:
                    nc.sync.dma_start(dbg_nov[p].rearrange("(t q) -> q t", q=128), P_nov[p][:])

            # =================== TOP-64 BATCH ===================
            novT = novT_pool.tile([PPC, N], F32, tag="novT")
            for p in range(PPC):
                for t in range(NT):
                    nc.sync.dma_start(
                        novT[p:p + 1, t * 128:(t + 1) * 128],
                        P_nov[p][:, t:t + 1])
            cur = novT
            for r in range(C // 8):
                nc.vector.max(out=cand[:, r * 8:(r + 1) * 8], in_=cur[:])
                if r < C // 8 - 1:
                    nxt = novT_pool.tile([PPC, N], F32, tag="novT")
                    nc.vector.match_replace(out=nxt[:], in_to_replace=cand[:, r * 8:(r + 1) * 8],
                                            in_values=cur[:], imm_value=NEG)
                    cur = nxt
            nc.vector.reduce_sum(csum[:], cand[:], axis=AX.X)
            if debug:
                nc.sync.dma_start(dbg_cand, cand[:])

            # =================== WRITE PHASE ===================
            for p in range(PPC):
                g_ap = P_gtdw[p][0:1, 0:1]
                tau_ap = P_gtdw[p][0:1, 1:2]
                dec_ap = P_gtdw[p][0:1, 2:3]
                ww_ap = P_gtdw[p][0:1, 3:4]

                candp0 = sm.tile([1, C], F32, tag="candp0")
                nc.sync.dma_start(candp0[:], cand[p:p + 1, :])
                csump0 = sm.tile([1, 1], F32, tag="csump0")
                nc.sync.dma_start(csump0[:], csum[p:p + 1, :])
                psB = poolX.tile([128, 130], F32, tag="tp")
                nc.tensor.matmul(psB[:128, 0:C], ones1x128[:], candp0[:],
                                 start=True, stop=True)
                candB = sm.tile([128, C], F32, tag="candB")
                nc.scalar.copy(candB[:], psB[:128, 0:C])
                psC1 = poolX.tile([128, 130], F32, tag="tp")
                nc.tensor.matmul(psC1[:C, 0:1], candp0[:], ones1x128[:, 0:1],
                                 start=True, stop=True)
                candcol = sm.tile([C, 1], F32, tag="candcol")
                nc.scalar.copy(candcol[:], psC1[:C, 0:1])

                ohT = rt1.tile([128, NT * C], F32, tag="q_nat")
                for i in range(NT):
                    nc.vector.tensor_scalar(
                        ohT[:, i * C:(i + 1) * C], candB[:], P_nov[p][:, i:i + 1], None,
                        op0=Alu.is_equal)
                psC = poolX.tile([128, 130], F32, tag="tp")
                for i in range(NT):
                    nc.tensor.matmul(psC[:C, 0:D], ohT[:, i * C:(i + 1) * C],
                                     P_qn[p][:, i * D:(i + 1) * D],
                                     start=(i == 0), stop=(i == NT - 1),
                                     skip_group_check=True)
                for i in range(NT):
                    nc.tensor.matmul(psC[:C, D:2 * D], ohT[:, i * C:(i + 1) * C],
                                     P_vn[p][:, i * D:(i + 1) * D],
                                     start=(i == 0), stop=(i == NT - 1),
                                     skip_group_check=True)
                ckv = sm.tile([C, 2 * D], F32, tag="ckv")
                nc.scalar.copy(ckv[:], psC[:C, 0:2 * D])
                if debug:
                    nc.sync.dma_start(dbg_ckv[p], ckv[:])

                sq = sm.tile([C, D], F32, tag="sqck")
                nrm2 = sm.tile([C, 1], F32, tag="nrm2")
                nc.vector.scalar_tensor_tensor(
                    out=sq[:], in0=ckv[:, 0:D], scalar=1.0, in1=ckv[:, 0:D],
                    op0=Alu.mult, op1=Alu.mult, accum_out=nrm2[:])
                rinv = sm.tile([C, 1], F32, tag="rinv")
                nc.vector.reciprocal(rinv[:], nrm2[:])
                rn = sm.tile([C, 1], F32, tag="rn")
                nc.scalar.activation(rn[:], rinv[:], Act.Sqrt)
                bl_rhs = sm.tile([C, 129], F32, tag="bl_rhs")
                nc.vector.tensor_scalar_mul(bl_rhs[:, 0:D], ckv[:, 0:D], rn[:])
                nc.scalar.copy(bl_rhs[:, D:2 * D], ckv[:, D:2 * D])
                nc.vector.memset(bl_rhs[:, 128:129], 1.0)

                ckT65 = sm.tile([65, C], F32, tag="ckT65")
                psT = poolX.tile([128, 130], F32, tag="tp")
                nc.tensor.transpose(psT[:D, 0:C], bl_rhs[:, 0:D], ident[:C, :C])
                nc.scalar.copy(ckT65[0:D, :], psT[:D, 0:C])
                negww = sm.tile([1, 1], F32, tag="negww")
                nc.vector.tensor_scalar_mul(negww[:], ww_ap, -1.0)
                psW = poolX.tile([128, 130], F32, tag="tp")
                nc.tensor.matmul(psW[:1, 0:C], negww[:], ones1x128[:, 0:C],
                                 start=True, stop=True)
                nc.scalar.copy(ckT65[64:65, :], psW[:1, 0:C])

                KTf = rtK.tile([65, M], F32, tag="KTf")
                for t in range(MT):
                    psT = poolX.tile([128, 130], F32, tag="tp")
                    nc.tensor.transpose(psT[:64, :128], P_emK[p][:, t * D:(t + 1) * D], ident[:])
                    nc.scalar.copy(KTf[0:64, t * 128:(t + 1) * 128], psT[:64, :128])
                sSrow = rt1.tile([1, M], F32, tag="sSrow")
                nc.sync.dma_start(sSrow[:], emS_d[p].rearrange("(a m) -> a m", a=1))
                nc.scalar.copy(KTf[64:65, :], sSrow[:])

                invtau1 = sm.tile([1, 1], F32, tag="invtau1")
                nc.vector.tensor_scalar_max(invtau1[:], tau_ap, 0.01)
                nc.vector.reciprocal(invtau1[:], invtau1[:])
                psI = poolX.tile([128, 130], F32, tag="tp")
                nc.tensor.matmul(psI[:C, 0:1], ones1x128[:, 0:C], invtau1[:],
                                 start=True, stop=True)
                invtau = sm.tile([C, 1], F32, tag="invtau")
                nc.scalar.copy(invtau[:], psI[:C, 0:1])

                slotraw = rtA.tile([C, M], F32, tag="bigA")
                for h in range(4):
                    psL = poolM.tile([128, 512], F32, tag="mmM")
                    nc.tensor.matmul(
                        psL[:C, :], ckT65[:], KTf[:, h * 512:(h + 1) * 512],
                        start=True, stop=True)
                    nc.scalar.copy(slotraw[:, h * 512:(h + 1) * 512], psL[:C, :])
                sw = rtC.tile([C, M], F32, tag="bigC")
                rmax = sm.tile([C, 1], F32, tag="rmax")
                nc.vector.reduce_max(rmax[:], slotraw[:], axis=AX.X)
                nbias = sm.tile([C, 1], F32, tag="nbias")
                nc.vector.tensor_tensor(out=nbias[:], in0=rmax[:], in1=invtau[:], op=Alu.mult)
                nc.vector.tensor_scalar_mul(nbias[:], nbias[:], -1.0)
                rsum = sm.tile([C, 1], F32, tag="rsum")
                nc.scalar.activation(sw[:], slotraw[:], Act.Exp, bias=nbias[:],
                                     scale=invtau[:], accum_out=rsum[:])

                gs1 = sm.tile([1, 1], F32, tag="gs1")
                nc.vector.tensor_scalar_add(gs1[:], csump0[:], EPS)
                nc.vector.reciprocal(gs1[:], gs1[:])
                nc.vector.tensor_tensor(out=gs1[:], in0=gs1[:], in1=g_ap, op=Alu.mult)
                psG = poolX.tile([128, 130], F32, tag="tp")
                nc.tensor.matmul(psG[:C, 0:1], ones1x128[:, 0:C], gs1[:], start=True, stop=True)
                gsC = sm.tile([C, 1], F32, tag="gsC")
                nc.scalar.copy(gsC[:], psG[:C, 0:1])
                alphacol = sm.tile([C, 1], F32, tag="alphacol")
                nc.vector.tensor_scalar_mul(alphacol[:], candcol[:], gsC[:])
                rr = sm.tile([C, 1], F32, tag="rr")
                nc.vector.reciprocal(rr[:], rsum[:])
                nc.vector.tensor_tensor(out=alphacol[:], in0=alphacol[:], in1=rr[:], op=Alu.mult)
                nc.vector.tensor_scalar_mul(sw[:], sw[:], alphacol[:])
                alpha = sw
                if debug:
                    nc.sync.dma_start(dbg_alpha[p], alpha[:])

                blKV = rtB2.tile([128, MT * 129], F32, tag="blKV")
                for t in range(MT):
                    psB2 = poolX.tile([128, 130], F32, tag="tp")
                    nc.tensor.matmul(psB2[:128, 0:129], alpha[:, t * 128:(t + 1) * 128],
                                     bl_rhs[:], start=True, stop=True)
                    nc.scalar.copy(blKV[:, t * 129:(t + 1) * 129], psB2[:128, 0:129])

                aps16 = sm.tile([128, MT], F32, tag="aps16")
                nc.vector.tensor_copy(
                    aps16[:], blKV[:].rearrange("q (t x) -> q t x", t=MT)[:, :, 128:129])
                masku = sm.tile([128, MT], F32, tag="masku")
                nc.vector.tensor_scalar(masku[:], aps16[:], EPS, None, op0=Alu.is_gt)
                blKview = blKV[:].rearrange("q (t x) -> q t x", t=MT)[:, :, 0:D]
                blVview = blKV[:].rearrange("q (t x) -> q t x", t=MT)[:, :, D:2 * D]
                sqb = rt1.tile([128, MT * D], F32, tag="sqb")
                nc.gpsimd.tensor_tensor(out=sqb[:].rearrange("q (t d) -> q t d", t=MT),
                                        in0=blKview, in1=blKview, op=Alu.mult)
                nrm2b = sm.tile([128, MT], F32, tag="nrm2b")
                nc.vector.reduce_sum(nrm2b[:], sqb[:].rearrange("q (t d) -> q t d", t=MT),
                                     axis=AX.X)
                nc.vector.tensor_scalar_max(nrm2b[:], nrm2b[:], 1e-30)
                rnb = sm.tile([128, MT], F32, tag="rnb")
                nc.vector.reciprocal(rnb[:], nrm2b[:])
                nc.scalar.activation(rnb[:], rnb[:], Act.Sqrt)
                nc.vector.tensor_tensor(out=rnb[:], in0=rnb[:], in1=masku[:], op=Alu.mult)
                aeff = sm.tile([128, MT], F32, tag="aeff")
                nc.vector.tensor_scalar_min(aeff[:], aps16[:], 1.0)
                nc.vector.tensor_tensor(out=aeff[:], in0=aeff[:], in1=masku[:], op=Alu.mult)
                onema = sm.tile([128, MT], F32, tag="onema")
                nc.vector.tensor_scalar(onema[:], aeff[:], -1.0, 1.0, op0=Alu.mult, op1=Alu.add)
                scalK = sm.tile([128, MT], F32, tag="scalK")
                nc.vector.tensor_tensor(out=scalK[:], in0=aeff[:], in1=rnb[:], op=Alu.mult)

                nKt = rtA.tile([128, MT * D], F32, tag="bigA")
                nVt = rtC.tile([128, MT * D], F32, tag="bigC")
                onema_b = onema[:].to_broadcast([128, MT, D])
                scalK_b = scalK[:].to_broadcast([128, MT, D])
                nKv = nKt[:].rearrange("q (t d) -> q t d", t=MT)
                nVv = nVt[:].rearrange("q (t d) -> q t d", t=MT)
                emKv = P_emK[p][:].rearrange("q (t d) -> q t d", t=MT)
                emVw = rt1.tile([128, MT * D], F32, tag="emVw")
                nc.sync.dma_start(emVw[:].rearrange("q (t d) -> q t d", t=MT),
                                  emV_d[p].rearrange("(t q) d -> q t d", q=128))
                emVv = emVw[:].rearrange("q (t d) -> q t d", t=MT)
                nc.gpsimd.tensor_tensor(out=nKv, in0=emKv, in1=onema_b, op=Alu.mult)
                sqb2 = rtC.tile([128, MT * D], F32, tag="bigC")
                nc.gpsimd.tensor_tensor(out=sqb2[:].rearrange("q (t d) -> q t d", t=MT),
                                        in0=blKview, in1=scalK_b, op=Alu.mult)
                nc.gpsimd.tensor_tensor(out=nKt[:], in0=nKt[:], in1=sqb2[:], op=Alu.add)
                nc.sync.dma_start(nK_d[p].rearrange("(t q) d -> q t d", q=128),
                                  nKt[:].rearrange("q (t d) -> q t d", t=MT))
                # new_V: scale = aeff / max(aps, eps)
                rdb = sm.tile([128, MT], F32, tag="rdb")
                nc.vector.tensor_scalar_max(rdb[:], aps16[:], EPS)
                nc.vector.reciprocal(rdb[:], rdb[:])
                nc.vector.tensor_tensor(out=rdb[:], in0=rdb[:], in1=aeff[:], op=Alu.mult)
                rdb_b = rdb[:].to_broadcast([128, MT, D])
                nc.gpsimd.tensor_tensor(out=nVv, in0=emVv, in1=onema_b, op=Alu.mult)
                nc.gpsimd.tensor_tensor(out=sqb[:].rearrange("q (t d) -> q t d", t=MT),
                                        in0=blVview, in1=rdb_b, op=Alu.mult)
                nc.gpsimd.tensor_tensor(out=nVt[:], in0=nVt[:], in1=sqb[:], op=Alu.add)
                nc.sync.dma_start(nV_d[p].rearrange("(t q) d -> q t d", q=128),
                                  nVt[:].rearrange("q (t d) -> q t d", t=MT))

                nS16 = sm.tile([128, MT], F32, tag="nS16")
                nc.vector.tensor_tensor(out=nS16[:], in0=P_S16[p][:], in1=aps16[:], op=Alu.add)
                nc.vector.tensor_scalar_min(nS16[:], nS16[:], S_MAX)
                nc.vector.tensor_scalar_max(nS16[:], nS16[:], 0.0)
                psD = poolX.tile([128, 130], F32, tag="tp")
                nc.tensor.matmul(psD[:128, 0:1], ones1x128[:], dec_ap, start=True, stop=True)
                dec128 = sm.tile([128, 1], F32, tag="dec128")
                nc.scalar.copy(dec128[:], psD[:128, 0:1])
                nc.vector.tensor_scalar_mul(nS16[:], nS16[:], dec128[:])
                colsum = sm.tile([128, 1], F32, tag="colsum")
                nc.vector.reduce_sum(colsum[:], nS16[:], axis=AX.X)
                psE = poolX.tile([128, 130], F32, tag="tp")
                nc.tensor.matmul(psE[:1, 0:1], colsum[:], ones128[:], start=True, stop=True)
                tot = sm.tile([1, 1], F32, tag="tot")
                nc.scalar.copy(tot[:], psE[:1, 0:1])
                nc.vector.tensor_scalar_add(tot[:], tot[:], EPS)
                nc.vector.reciprocal(tot[:], tot[:])
                nc.vector.tensor_scalar(tot[:], tot[:], BUDGET, 1.0, op0=Alu.mult, op1=Alu.min)
                psF = poolX.tile([128, 130], F32, tag="tp")
                nc.tensor.matmul(psF[:128, 0:1], ones1x128[:], tot[:], start=True, stop=True)
                sc128 = sm.tile([128, 1], F32, tag="sc128")
                nc.scalar.copy(sc128[:], psF[:128, 0:1])
                nc.vector.tensor_scalar_mul(nS16[:], nS16[:], sc128[:])
                nc.sync.dma_start(nS_d[p].rearrange("(t q) -> q t", q=128), nS16[:])

                nA16 = sm.tile([128, MT], F32, tag="nA16")
                nc.vector.tensor_scalar(nA16[:], aps16[:], -1.0, 1.0, op0=Alu.mult, op1=Alu.add)
                nc.vector.tensor_tensor(out=nA16[:], in0=nA16[:], in1=P_A16[p][:], op=Alu.mult)
                nc.sync.dma_start(nA_d[p].rearrange("(t q) -> q t", q=128), nA16[:])

    nc.compile()
    return nc


_CACHE = {}


def get_program(debug=False):
    key = bool(debug)
    if key not in _CACHE:
        _CACHE[key] = build_program(debug=debug)
    return _CACHE[key]


def shard_inputs(inputs):
    q = np.ascontiguousarray(np.asarray(inputs['q']).transpose(0, 2, 1, 3).reshape(PAIRS, N, D))
    qn = np.ascontiguousarray(np.asarray(inputs['q_nov']).transpose(0, 2, 1, 3).reshape(PAIRS, N, D))
    vn = np.ascontiguousarray(np.asarray(inputs['v_nov']).transpose(0, 2, 1, 3).reshape(PAIRS, N, D))
    sur = np.ascontiguousarray(np.asarray(inputs['surprise']).transpose(0, 2, 1).reshape(PAIRS, N))
    wn = np.ascontiguousarray(np.asarray(inputs['w_nov']).transpose(0, 2, 1).reshape(PAIRS, N))
    gtdw = np.ascontiguousarray(
        np.stack([np.asarray(inputs['g_em']), np.asarray(inputs['tau']),
                  np.asarray(inputs['decay']), np.asarray(inputs['ww'])], axis=-1
                 ).reshape(PAIRS, 4).astype(np.float32))
    emK = np.ascontiguousarray(np.asarray(inputs['em_K']).reshape(PAIRS, M, D))
    emV = np.ascontiguousarray(np.asarray(inputs['em_V']).reshape(PAIRS, M, D))
    emS = np.ascontiguousarray(np.asarray(inputs['em_S']).reshape(PAIRS, M))
    emA = np.ascontiguousarray(np.asarray(inputs['em_age']).reshape(PAIRS, M))
    in_maps = []
    for c in range(NCORES):
        s = slice(c * PPC, (c + 1) * PPC)
        in_maps.append({
            "q": q[s], "qn": qn[s], "vn": vn[s], "sur": sur[s], "wn": wn[s],
            "gtdw": gtdw[s], "emK": emK[s], "emV": emV[s], "emS": emS[s],
            "emA": emA[s],
        })
    return in_maps


def unshard_outputs(results):
    def cat(name):
        return np.concatenate([r[name] for r in results], axis=0)
    out = cat("out").reshape(BS, B, N, D).transpose(0, 2, 1, 3)
    nK = cat("nK").reshape(BS, B, M, D)
    nV = cat("nV").reshape(BS, B, M, D)
    nS = cat("nS").reshape(BS, B, M)
    nA = cat("nA").reshape(BS, B, M)
    return (np.ascontiguousarray(out), nK, nV, nS, nA)


def kernel(**inputs):
    assert int(inputs.get('C_cand', C)) == C
    nc = get_program(debug=False)
    in_maps = shard_inputs(inputs)
    res = run_bass_kernel_spmd(nc, in_maps, core_ids=list(range(NCORES)))
    return unshard_outputs(res.results)
